# revision 1
# baseline (speedup 1.0000x reference)
"""AgentAttention TRN2 kernel: 8 cores = 4 batches x 2 head-groups.

Reference computation (B=4, T=3584, dim=1024, H=16, D=64, P=7):
  qkv = x @ W_qkv -> q,k,v [B,H,T,D]
  agent = avgpool_T(q) [B,H,P,D]
  v_agent = softmax(agent*SC @ k^T) @ v
  out_att = softmax(q*SC @ agent^T) @ v_agent
  dwc = depthwise3x3 over (H,T) of v
  out = (out_att + dwc) 'b h t d -> b t (h d)' @ W_o + b_o

Core c handles batch c//2, heads [8g, 8g+8) with g=c%2. W_qkv column-split
(with v halo head columns, zeroed outside [0,16)), W_o row-split; the two
partial outputs per batch are summed on the host (+ b_o).

Device-side layout strategy: everything transposed (feature dims on SBUF
partitions). x is PE-transposed to xT; one bf16 matmul produces qT/kT/vT;
agent pooling is a free-dim reduce of the q psum; both softmaxes skip
max-subtraction (scores are provably tiny); stage-1 aggregation is
re-associated through x ((u1^T @ x) @ Wv) to avoid needing v in natural
layout; the depthwise conv runs as diagonal-matrix matmuls on fp32r values
with v stored in both even/odd head-pair alignments; the projection is an
fp32r matmul from the Y accumulator.
"""

import numpy as np
import ml_dtypes

import concourse.bass as bass
import concourse.bacc as bacc
import concourse.mybir as mybir
import concourse.tile as tile
from concourse.bass import ts, ds
from concourse import bass_utils

F32 = mybir.dt.float32
F32R = mybir.dt.float32r
BF16 = mybir.dt.bfloat16
AX = mybir.AxisListType
AF = mybir.ActivationFunctionType

T, DIM, D, P = 3584, 1024, 64, 7
HL = 8                      # local heads per core
SC = D ** -0.5
NT = T // 128               # 28 token tiles of 128
NCH = T // 512              # 7 chunks of 512
TPAD = T + 2                # vT padded with one zero col each side


def build_nc(skip=()):
    # Bacc (not plain Bass): its compile() runs generate_event_semaphores,
    # which splits multi-wait sync_info into InstEventSemaphore -- TRN2
    # instructions can carry at most one embedded wait.
    nc = bacc.Bacc("TRN2", target_bir_lowering=False)

    xb = nc.dram_tensor("xb", [T, DIM], BF16, kind="ExternalInput")
    wcat = nc.dram_tensor("wcat", [DIM, 1664], BF16, kind="ExternalInput")
    wvloc = nc.dram_tensor("wvloc", [DIM, 512], BF16, kind="ExternalInput")
    wo = nc.dram_tensor("wo", [512, DIM], F32R, kind="ExternalInput")
    taps = nc.dram_tensor("taps", [9, 128, 128], F32R, kind="ExternalInput")
    convb2 = nc.dram_tensor("convb2", [128, 1], F32, kind="ExternalInput")
    idf = nc.dram_tensor("idf", [128, 128], F32, kind="ExternalInput")
    idb = nc.dram_tensor("idb", [128, 128], BF16, kind="ExternalInput")
    outp = nc.dram_tensor("outp", [T, DIM], F32, kind="ExternalOutput")

    with tile.TileContext(nc) as tc:
        _emit(nc, tc, xb, wcat, wvloc, wo, taps, convb2, idf, idb, outp,
              frozenset(skip))
    nc.compile()
    return nc


def _emit(nc, tc, xb, wcat, wvloc, wo, taps, convb2, idf, idb, outp, skip):
    import contextlib
    ctx = contextlib.ExitStack()
    with ctx:
        # ---- persistent small constants -------------------------------
        pconst = ctx.enter_context(tc.tile_pool(name="const", bufs=1))
        idf_sb = pconst.tile([128, 128], F32, name="idf", tag="idf")
        nc.sync.dma_start(idf_sb[:], idf[:])
        idb_sb = pconst.tile([128, 128], BF16, name="idb", tag="idb")
        nc.sync.dma_start(idb_sb[:], idb[:])
        cb_sb = pconst.tile([128, 1], F32, name="cb", tag="cb")
        nc.sync.dma_start(cb_sb[:], convb2[:])
        ones_sb = pconst.tile([128, 1], BF16, name="ones", tag="ones")
        nc.vector.memset(ones_sb[:], 1.0)

        psmall = ctx.enter_context(tc.tile_pool(name="small", bufs=1))
        agT = [psmall.tile([128, P], F32, name=f"agT{j}", tag=f"agT{j}")
               for j in range(4)]
        R = [psmall.tile([128, 2 * P], BF16, name=f"R{j}", tag=f"R{j}")
             for j in range(4)]
        u1T = psmall.tile([128, NT * 56], BF16, name="u1T", tag="u1T")
        p2T = psmall.tile([56, T], BF16, name="p2T", tag="p2T")
        vabd = psmall.tile([56, 512], BF16, name="vabd", tag="vabd")
        rec1 = psmall.tile([56, 1], F32, name="rec1", tag="rec1")

        # ---- vT: v (10 head slots incl halo) transposed, f32r, t-padded
        pvT = ctx.enter_context(tc.tile_pool(name="vT", bufs=1))
        vT = [pvT.tile([128, TPAD], F32R, name=f"vT{j}", tag=f"vT{j}")
              for j in range(5)]
        for j in range(5):
            nc.vector.memset(vT[j][:, 0:1].bitcast(F32), 0.0)
            nc.vector.memset(vT[j][:, TPAD - 1:TPAD].bitcast(F32), 0.0)

        # ---- phase 1: x PE-transpose + qkv matmul ---------------------
        import contextlib as _cl
        qk_stack = _cl.ExitStack()
        pqT = qk_stack.enter_context(tc.tile_pool(name="qT", bufs=1))
        qT = [pqT.tile([128, T], BF16, name=f"qT{j}", tag=f"qT{j}")
              for j in range(4)]
        pkT = qk_stack.enter_context(tc.tile_pool(name="kT", bufs=1))
        kT = [pkT.tile([128, T], BF16, name=f"kT{j}", tag=f"kT{j}")
              for j in range(4)]

        if "phase1" not in skip:
            _phase1(nc, tc, xb, wcat, idb_sb, qT, kT, vT, agT)

        # R = agentT * SC/512 as 4 block tiles [128, 14]
        for j in range(4):
            nc.vector.memset(R[j][:], 0.0)
            nc.scalar.activation(
                R[j][0:64, 0:P], agT[j][0:64, :], AF.Copy, scale=SC / 512.0)
            nc.scalar.activation(
                R[j][64:128, P:2 * P], agT[j][64:128, :], AF.Copy,
                scale=SC / 512.0)

        # ---- scores: s1 (agents<-keys), s2 (queries<-agents) ----------
        if "scores" in skip:
            nc.vector.memset(u1T[:], 0.0)
            nc.vector.memset(p2T[:], 0.0)
        else:
            _scores(nc, tc, qT, kT, R, u1T, p2T, idf_sb)

        qk_stack.close()  # free qT/kT SBUF

        # taps + vO early: the SBUF->SBUF partition-shift DMAs overlap the
        # scores section. vO holds odd-aligned slot pairs so every conv tap
        # is a full-128 aligned matmul; Y[i] later reuses vO[i]'s slot
        # (same pool tag), which requires all 7 chunk psums live (bufs=7).
        ptaps = ctx.enter_context(tc.tile_pool(name="taps", bufs=1))
        taps_sb = ptaps.tile([128, 9, 128], F32R, name="taps", tag="taps")
        for k9 in range(9):
            nc.sync.dma_start(taps_sb[:, k9, :], taps[k9])
        pvO = ctx.enter_context(tc.tile_pool(name="vO", bufs=1))
        vO = [pvO.tile([128, TPAD], F32R, name=f"vO{i}", tag=f"vO{i}")
              for i in range(4)]
        for i in range(4):
            nc.sync.dma_start(vO[i][0:64, :], vT[i][64:128, :])
            nc.sync.dma_start(vO[i][64:128, :], vT[i + 1][0:64, :])
        wv_sb = pvO.tile([128, 8, 512], BF16, name="wv", tag="vO0")
        for kk in range(8):
            nc.sync.dma_start(wv_sb[:, kk, :], wvloc[ts(kk, 128), :])


        # ---- dwc (depthwise 3x3 over (head, t)) -> Y ------------------
        pY = ctx.enter_context(tc.tile_pool(name="Ypool", bufs=1))

        # agg pools stay open across the interleaved dwc/agg emission
        agg_on = "agg" not in skip
        agg_stack = _cl.ExitStack()
        if agg_on:
            pxn = agg_stack.enter_context(tc.tile_pool(name="xnat", bufs=4))


            pa1p = agg_stack.enter_context(
                tc.tile_pool(name="a1ps", bufs=1, space="PSUM"))
            csp = agg_stack.enter_context(
                tc.tile_pool(name="csps", bufs=1, space="PSUM"))
            pa1 = [pa1p.tile([56, DIM], F32, name=f"a1_{e}", tag=f"a1_{e}")
                   for e in range(2)]
            pcs = csp.tile([56, 1], F32, name="cs", tag="cs")

            def agg_slice(lo, hi):
                for tt in range(lo, hi):
                    xn = pxn.tile([128, DIM], BF16, name="xn", tag="xn")
                    nc.scalar.dma_start(xn[:], xb[ts(tt, 128), :])
                    e = tt % 2
                    for half in range(2):
                        nc.tensor.matmul(
                            pa1[e][:, ts(half, 512)], u1T[:, ts(tt, 56)],
                            xn[:, ts(half, 512)],
                            start=(tt == e), stop=(tt >= NT - 2))
                    nc.tensor.matmul(
                        pcs[:], u1T[:, ts(tt, 56)], ones_sb[:],
                        start=(tt == 0), stop=(tt == NT - 1))
        else:
            def agg_slice(lo, hi):
                pass

        Y = []
        with tc.tile_pool(name="dwcps", bufs=3, space="PSUM") as pdw:
            for i in range(4):
                src_by_kh = (vT[i], vO[i], vT[i + 1])
                pds = []
                for tc7 in range(NCH):
                    off = 1 + tc7 * 512
                    pd = pdw.tile([128, 512], F32, name="dwc", tag="dwc")
                    pds.append(pd)
                    if "dwc" not in skip:
                        n = 0
                        for kh in range(3):
                            for kt in range(3):
                                nc.tensor.matmul(
                                    pd[:], taps_sb[:, kh * 3 + kt, :],
                                    src_by_kh[kh][:, ds(off + kt - 1, 512)],
                                    start=(n == 0), stop=(n == 8))
                                n += 1
                    else:
                        nc.tensor.matmul(
                            pd[:], taps_sb[:, 0, :],
                            vT[i][:, ds(off, 512)], start=True, stop=True)
                    agg_slice(7 * i + tc7, 7 * i + tc7 + 1)
                Yi = pY.tile([128, T], F32R, name=f"Y{i}", tag=f"Y{i}")
                Y.append(Yi)
                for tc7 in range(NCH):
                    if tc7 % 2 == 0:
                        nc.scalar.activation(
                            Yi[:, ts(tc7, 512)], pds[tc7][:], AF.Identity,
                            bias=cb_sb[:, 0:1])
                    else:
                        nc.vector.tensor_scalar(
                            out=Yi[:, ts(tc7, 512)], in0=pds[tc7][:],
                            scalar1=cb_sb[:, 0:1], scalar2=None,
                            op0=mybir.AluOpType.add)

        if agg_on:
            with tc.tile_pool(name="vaps", bufs=1, space="PSUM") as pvap, \
                 tc.tile_pool(name="a2tps", bufs=2, space="PSUM") as pa2t:
                nc.vector.reciprocal(rec1[:], pcs[:])
                a2 = pvO.tile([56, DIM], BF16, name="a2", tag="vO1")
                nc.scalar.copy(a2[:], pa1[0][:])
                nc.vector.tensor_add(a2[:], a2[:], pa1[1][:])
                pva = pvap.tile([56, 512], F32, name="va", tag="va")
                a2ts_all = pvO.tile([128, 8, 56], BF16, name="a2ts_all",
                                    tag="vO3")
                for kk in range(8):
                    pt = pa2t.tile([128, 56], BF16, name="a2t", tag="a2t")
                    nc.tensor.transpose(
                        pt[:], a2[:, ts(kk, 128)], idb_sb[0:56, 0:56])
                    nc.any.tensor_copy(a2ts_all[:, kk, :], pt[:])
                    nc.tensor.matmul(pva[:], a2ts_all[:, kk, :],
                                     wv_sb[:, kk, :],
                                     start=(kk == 0), stop=(kk == 7))
                van = pvO.tile([56, 512], BF16, name="van", tag="vO2")
                nc.vector.tensor_scalar(
                    out=van[:], in0=pva[:], scalar1=rec1[:],
                    scalar2=None, op0=mybir.AluOpType.mult,
                )
                nc.vector.memset(vabd[:], 0.0)
                for h in range(HL):
                    nc.sync.dma_start(
                        vabd[ds(P * h, P), ds(64 * h, 64)],
                        van[ds(P * h, P), ds(64 * h, 64)])
        else:
            nc.vector.memset(vabd[:], 0.0)
        agg_stack.close()

        # ---- attention output: Y += vabd^T @ p2T ----------------------
        with tc.tile_pool(name="attps", bufs=3, space="PSUM") as pat:
            for tc7 in range(NCH):
                for i in range(4):
                    pa = pat.tile([128, 512], F32, name="att", tag="att")
                    nc.tensor.matmul(pa[:], vabd[:, ts(i, 128)],
                                     p2T[:, ts(tc7, 512)],
                                     start=True, stop=True)
                    nc.vector.tensor_add(
                        Y[i][:, ts(tc7, 512)], Y[i][:, ts(tc7, 512)], pa[:])

        # ---- output projection: out = Y^T @ Wo ------------------------
        with tc.tile_pool(name="ostage", bufs=2, ) as pos, \
             tc.tile_pool(name="ops", bufs=4, space="PSUM") as pop:
            wo_sb = [pvO.tile([128, DIM], F32R, name=f"wo{k}", tag=f"vO{k}")
                     for k in range(4)]
            for k in range(4):
                nc.scalar.dma_start(wo_sb[k][:], wo[ts(k, 128), :])
            for tt in range(NT):
                po = pop.tile([128, DIM], F32, name="o", tag="o")
                for half in range(2):
                    for k in range(4):
                        nc.tensor.matmul(
                            po[:, ts(half, 512)],
                            Y[k][:, ts(tt, 128)],
                            wo_sb[k][:, ts(half, 512)],
                            start=(k == 0), stop=(k == 3))
                osta = pos.tile([128, 512], F32, name="osta", tag="osta")
                ostb = pos.tile([128, 512], F32, name="ostb", tag="ostb")
                nc.vector.tensor_copy(osta[:], po[:, 0:512])
                nc.scalar.copy(ostb[:], po[:, 512:DIM])
                nc.scalar.dma_start(outp[ts(tt, 128), 0:512], osta[:])
                nc.sync.dma_start(outp[ts(tt, 128), 512:DIM], ostb[:])


def _copy(eng, out, in_):
    if eng.__class__.__name__ == "BassScalarEngine" or hasattr(eng, "activation"):
        eng.copy(out, in_)
    else:
        eng.tensor_copy(out, in_)


def _phase1(nc, tc, xb, wcat, idb_sb, qT, kT, vT, agT):
    with tc.tile_pool(name="xT", bufs=1) as pxT, \
         tc.tile_pool(name="xload", bufs=3) as pxl, \
         tc.tile_pool(name="wstream", bufs=3) as pw, \
         tc.tile_pool(name="tpps", bufs=4, space="PSUM") as ptp, \
         tc.tile_pool(name="mmps", bufs=4, space="PSUM") as pmm:
        xTb = pxT.tile([128, 8 * T], BF16, name="xTb", tag="xTb")

        def xT(j):
            return xTb[:, ds(j * T, T)]

        for tt in range(NT):
            xn = pxl.tile([128, DIM], BF16, name="xn0", tag="xn0")
            nc.sync.dma_start(xn[:], xb[ts(tt, 128), :])
            for grp in range(2):
                pp = ptp.tile([128, 512], BF16, name="tp", tag="tp")
                for q in range(4):
                    nc.tensor.matmul(
                        pp[:, ts(q, 128)], xn[:, ts(grp * 4 + q, 128)],
                        idb_sb[:], is_transpose=True,
                        start=(q == 0), stop=(q == 3),
                        skip_group_check=True)
                nc.any.tensor_copy(
                    xTb.rearrange("p (j t) -> p j t", t=T)
                       [:, ds(grp * 4, 4), ts(tt, 128)],
                    pp.rearrange("p (q n) -> p q n", n=128))

        for cg in range(13):
            wt = pw.tile([128, 8, 128], BF16, name="w", tag="w")
            for kk in range(8):
                nc.sync.dma_start(
                    wt[:, kk, :], wcat[ts(kk, 128), ts(cg, 128)])
            for ch in range(NCH):
                pm = pmm.tile([128, 512], F32, name="mm", tag="mm")
                for kk in range(8):
                    nc.tensor.matmul(
                        pm[:], wt[:, kk, :], xT(kk)[:, ds(ch * 512, 512)],
                        start=(kk == 0), stop=(kk == 7),
                    )
                eng = nc.scalar if (cg * NCH + ch) % 2 else nc.vector
                if cg < 4:        # q columns
                    _copy(eng, qT[cg][:, ts(ch, 512)], pm[:])
                    nc.vector.reduce_sum(
                        agT[cg][:, ch:ch + 1], pm[:], axis=AX.X)
                elif cg < 8:      # k columns
                    _copy(eng, kT[cg - 4][:, ts(ch, 512)], pm[:])
                else:             # v columns (10 slots incl halo)
                    _copy(eng, vT[cg - 8][:, ds(1 + ch * 512, 512)], pm[:])


def _scores(nc, tc, qT, kT, R, u1T, p2T, idf_sb):
    with tc.tile_pool(name="sps", bufs=2, space="PSUM") as pps, \
         tc.tile_pool(name="trps", bufs=2, space="PSUM") as ptr, \
         tc.tile_pool(name="stmp", bufs=3) as pst:
        for tt in range(NT):
            ps1 = pps.tile([128, 56], F32, name="s1", tag="s1")
            for j in range(4):
                nc.tensor.matmul(
                    ps1[:, ts(j, 14)], kT[j][:, ts(tt, 128)], R[j][:],
                    start=(j == 0), stop=(j == 3), skip_group_check=True,
                )
            nc.scalar.activation(u1T[:, ts(tt, 56)], ps1[:], AF.Exp)

            ps2 = pps.tile([128, 56], F32, name="s2", tag="s2")
            for j in range(4):
                nc.tensor.matmul(
                    ps2[:, ts(j, 14)], qT[j][:, ts(tt, 128)], R[j][:],
                    start=(j == 0), stop=(j == 3), skip_group_check=True,
                )
            u2 = pst.tile([128, 56], F32, name="u2", tag="u2")
            nc.scalar.activation(u2[:], ps2[:], AF.Exp)
            rs = pst.tile([128, 8], F32, name="rs", tag="rs")
            nc.vector.reduce_sum(
                rs[:], u2.rearrange("p (h q) -> p h q", q=P), axis=AX.X)
            nc.vector.reciprocal(rs[:], rs[:])
            p2f = pst.tile([128, 56], F32, name="p2f", tag="p2f")
            nc.vector.tensor_tensor(
                out=p2f.rearrange("p (h q) -> p h q", q=P),
                in0=u2.rearrange("p (h q) -> p h q", q=P),
                in1=rs[:, :, None].broadcast_to([128, 8, P]),
                op=mybir.AluOpType.mult,
            )
            ptt = ptr.tile([56, 128], F32, name="p2t", tag="p2t")
            nc.tensor.transpose(ptt[:], p2f[:], idf_sb[:])
            nc.any.tensor_copy(p2T[:, ts(tt, 128)], ptt[:])


def _agg(nc, tc, xb, wvloc, u1T, ones_sb, idb_sb, rec1, vabd):
    with tc.tile_pool(name="xnat", bufs=8) as pxn, \
         tc.tile_pool(name="wv", bufs=1) as pwv, \
         tc.tile_pool(name="aggtmp", bufs=1) as pag, \
         tc.tile_pool(name="a1ps", bufs=1, space="PSUM") as pa1p, \
         tc.tile_pool(name="csps", bufs=1, space="PSUM") as pcsp, \
         tc.tile_pool(name="vaps", bufs=1, space="PSUM") as pvap, \
         tc.tile_pool(name="a2tps", bufs=2, space="PSUM") as pa2t:
        pa1 = [pa1p.tile([56, DIM], F32, name=f"a1_{e}", tag=f"a1_{e}")
               for e in range(2)]
        pcs = pcsp.tile([56, 1], F32, name="cs", tag="cs")
        for tt in range(NT):
            xn = pxn.tile([128, DIM], BF16, name="xn", tag="xn")
            nc.scalar.dma_start(xn[:], xb[ts(tt, 128), :])
            e = tt % 2
            for half in range(2):
                nc.tensor.matmul(
                    pa1[e][:, ts(half, 512)], u1T[:, ts(tt, 56)],
                    xn[:, ts(half, 512)],
                    start=(tt == e), stop=(tt >= NT - 2))
            nc.tensor.matmul(pcs[:], u1T[:, ts(tt, 56)], ones_sb[:],
                             start=(tt == 0), stop=(tt == NT - 1))
        nc.vector.reciprocal(rec1[:], pcs[:])
        a2 = pag.tile([56, DIM], BF16, name="a2", tag="a2")
        nc.scalar.copy(a2[:], pa1[0][:])
        nc.vector.tensor_add(a2[:], a2[:], pa1[1][:])

        wv_sb = pwv.tile([128, 8, 512], BF16, name="wv", tag="wv")
        for kk in range(8):
            nc.sync.dma_start(wv_sb[:, kk, :], wvloc[ts(kk, 128), :])
        pva = pvap.tile([56, 512], F32, name="va", tag="va")
        for kk in range(8):
            pt = pa2t.tile([128, 56], BF16, name="a2t", tag="a2t")
            nc.tensor.transpose(
                pt[:], a2[:, ts(kk, 128)], idb_sb[0:56, 0:56])
            a2t = pag.tile([128, 56], BF16, name=f"a2ts{kk}", tag=f"a2ts{kk}")
            nc.any.tensor_copy(a2t[:], pt[:])
            nc.tensor.matmul(pva[:], a2t[:], wv_sb[:, kk, :],
                             start=(kk == 0), stop=(kk == 7))
        van = pag.tile([56, 512], BF16, name="van", tag="van")
        nc.vector.tensor_scalar(
            out=van[:], in0=pva[:], scalar1=rec1[:],
            scalar2=None, op0=mybir.AluOpType.mult,
        )
        nc.vector.memset(vabd[:], 0.0)
        for h in range(HL):
            nc.sync.dma_start(
                vabd[ds(P * h, P), ds(64 * h, 64)],
                van[ds(P * h, P), ds(64 * h, 64)])


_NC_CACHE = None


def _get_nc():
    global _NC_CACHE
    if _NC_CACHE is None:
        _NC_CACHE = build_nc()
    return _NC_CACHE


def _prep_core_inputs(x, W_qkv, W_o, conv_w):
    bf = ml_dtypes.bfloat16
    ins = []
    idf_np = np.eye(128, dtype=np.float32)
    idb_np = np.eye(128, dtype=bf)
    # taps[kh*3+kt] = kron(I2, diag(conv_w[:, 0, kh, kt]))
    taps_np = np.zeros((9, 128, 128), dtype=np.float32)
    cw = np.asarray(conv_w, np.float32)
    for kh in range(3):
        for kt in range(3):
            dg = np.diag(cw[:, 0, kh, kt])
            taps_np[kh * 3 + kt, 0:64, 0:64] = dg
            taps_np[kh * 3 + kt, 64:128, 64:128] = dg
    for c in range(8):
        b, g = c // 2, c % 2
        wq = W_qkv[:, 512 * g:512 * g + 512]
        wk = W_qkv[:, 1024 + 512 * g:1024 + 512 * g + 512]
        wv10 = np.zeros((DIM, 640), np.float32)
        for s in range(10):
            h = 8 * g - 1 + s
            if 0 <= h < 16:
                wv10[:, 64 * s:64 * s + 64] = \
                    W_qkv[:, 2048 + 64 * h:2048 + 64 * h + 64]
        wcat = np.concatenate([wq, wk, wv10], axis=1)
        ins.append({
            "xb": np.ascontiguousarray(x[b]).astype(bf),
            "wcat": np.ascontiguousarray(wcat).astype(bf),
            "wvloc": np.ascontiguousarray(wv10[:, 64:576]).astype(bf),
            "wo": np.ascontiguousarray(
                W_o[512 * g:512 * g + 512, :], np.float32),
            "taps": taps_np,
            "convb2": np.zeros((128, 1), np.float32),
            "idf": idf_np,
            "idb": idb_np,
        })
    return ins


def kernel(x, W_qkv, W_o, b_o, conv_w, conv_b, _run_kwargs=None):
    x = np.asarray(x, np.float32)
    W_qkv = np.asarray(W_qkv, np.float32)
    W_o = np.asarray(W_o, np.float32)
    b_o = np.asarray(b_o, np.float32)
    conv_w = np.asarray(conv_w, np.float32)
    conv_b = np.asarray(conv_b, np.float32)

    ins = _prep_core_inputs(x, W_qkv, W_o, conv_w)
    cb2 = np.tile(conv_b, 2).astype(np.float32).reshape(128, 1)
    for m in ins:
        m["convb2"] = cb2

    nc = _get_nc()
    res = bass_utils.run_bass_kernel_spmd(
        nc, ins, core_ids=list(range(8)), **(_run_kwargs or {}))
    outs = [r["outp"] for r in res.results]
    B = x.shape[0]
    full = np.empty((B, T, DIM), np.float32)
    for b in range(B):
        full[b] = outs[2 * b] + outs[2 * b + 1] + b_o[None, :]
    if _run_kwargs:
        kernel.last_results = res
    return full



# revision 7
# speedup vs baseline: 1.2713x; 1.2713x over previous
"""AgentAttention TRN2 kernel: 8 cores = 4 batches x 2 head-groups.

Reference computation (B=4, T=3584, dim=1024, H=16, D=64, P=7):
  qkv = x @ W_qkv -> q,k,v [B,H,T,D]
  agent = avgpool_T(q) [B,H,P,D]
  v_agent = softmax(agent*SC @ k^T) @ v
  out_att = softmax(q*SC @ agent^T) @ v_agent
  dwc = depthwise3x3 over (H,T) of v
  out = (out_att + dwc) 'b h t d -> b t (h d)' @ W_o + b_o

Core c handles batch c//2, heads [8g, 8g+8) with g=c%2. The two partial
outputs per batch are summed on the host (+ b_o).

Key restructuring vs the straightforward version: q and k are never
materialized. Both score matrices contract against x directly:
  s1^T = x @ A^T  with  A = (agent*SC) @ Wk^T     [T, 56]
  s2   = x @ B    with  B = Wq @ (agent^T*SC)     [T, 56]
agent itself comes from pooled x (agent = xsum @ Wq, xsum = chunk sums
of x computed on-device by free-dim reduces of x^T). x^T is supplied
pre-transposed by the host, so the only big PE matmul on the qkv side
is v (10 head slots incl conv halo, 640 cols). Stage-1 aggregation is
re-associated through x ((u1^T @ x) @ Wv). The stage-2 attention
output matmul accumulates directly into the depthwise-conv PSUM tile
(a 10th accumulation step), so Y is extracted once with the conv bias.
v/taps/Y/W_o are bf16 (PE rate identical to f32r, half the SBUF/DMA).
"""

import numpy as np
import ml_dtypes

import concourse.bass as bass
import concourse.bacc as bacc
import concourse.mybir as mybir
import concourse.tile as tile
from concourse.bass import ts, ds
from concourse import bass_utils

F32 = mybir.dt.float32
F32R = mybir.dt.float32r
BF16 = mybir.dt.bfloat16
AX = mybir.AxisListType
AF = mybir.ActivationFunctionType
AL = mybir.AluOpType

T, DIM, D, P = 3584, 1024, 64, 7
HL = 8                      # local heads per core
SC = D ** -0.5
NT = T // 128               # 28 token tiles of 128
NCH = T // 512              # 7 chunks of 512
TPAD = T + 2                # vT padded with one zero col each side


def build_nc(skip=()):
    # Bacc (not plain Bass): its compile() runs generate_event_semaphores,
    # which splits multi-wait sync_info into InstEventSemaphore -- TRN2
    # instructions can carry at most one embedded wait.
    nc = bacc.Bacc("TRN2", target_bir_lowering=False)

    xbT = nc.dram_tensor("xbT", [DIM, T], BF16, kind="ExternalInput")
    xb = nc.dram_tensor("xb", [T, DIM], BF16, kind="ExternalInput")
    wv10 = nc.dram_tensor("wv10", [DIM, 640], BF16, kind="ExternalInput")
    wvloc = nc.dram_tensor("wvloc", [DIM, 512], BF16, kind="ExternalInput")
    wqn = nc.dram_tensor("wqn", [DIM, 512], BF16, kind="ExternalInput")
    wkt = nc.dram_tensor("wkt", [512, DIM], BF16, kind="ExternalInput")
    wqt = nc.dram_tensor("wqt", [512, DIM], BF16, kind="ExternalInput")
    wo = nc.dram_tensor("wo", [512, DIM], BF16, kind="ExternalInput")
    taps = nc.dram_tensor("taps", [9, 128, 128], BF16, kind="ExternalInput")
    convb2 = nc.dram_tensor("convb2", [128, 1], F32, kind="ExternalInput")
    idf = nc.dram_tensor("idf", [128, 128], F32, kind="ExternalInput")
    idb = nc.dram_tensor("idb", [128, 128], BF16, kind="ExternalInput")
    outp = nc.dram_tensor("outp", [T, DIM], F32, kind="ExternalOutput")

    with tile.TileContext(nc) as tc:
        _emit(nc, tc, xbT, xb, wv10, wvloc, wqn, wkt, wqt, wo, taps, convb2,
              idf, idb, outp)
    nc.compile()
    return nc


def _copy(eng, out, in_):
    if hasattr(eng, "activation"):
        eng.copy(out, in_)
    else:
        eng.tensor_copy(out, in_)


def _emit(nc, tc, xbT, xb, wv10, wvloc, wqn, wkt, wqt, wo, taps, convb2,
          idf, idb, outp):
    import contextlib
    ctx = contextlib.ExitStack()
    with ctx:
        # ---- persistent small constants -------------------------------
        pconst = ctx.enter_context(tc.tile_pool(name="const", bufs=1))
        idf_sb = pconst.tile([128, 128], F32, name="idf", tag="idf")
        nc.sync.dma_start(idf_sb[:], idf[:])
        idb_sb = pconst.tile([128, 128], BF16, name="idb", tag="idb")
        nc.sync.dma_start(idb_sb[:], idb[:])
        cb_sb = pconst.tile([128, 1], F32, name="cb", tag="cb")
        nc.sync.dma_start(cb_sb[:], convb2[:])
        ones_sb = pconst.tile([128, 1], BF16, name="ones", tag="ones")
        nc.vector.memset(ones_sb[:], 1.0)

        psmall = ctx.enter_context(tc.tile_pool(name="small", bufs=1))
        xsumT = [psmall.tile([128, P], F32, name=f"xsT{k}", tag=f"xsT{k}")
                 for k in range(8)]
        xsumB = [psmall.tile([128, P], BF16, name=f"xsB{k}", tag=f"xsB{k}")
                 for k in range(8)]
        R = [psmall.tile([128, 2 * P], BF16, name=f"R{j}", tag=f"R{j}")
             for j in range(4)]
        AB = [psmall.tile([128, 112], BF16, name=f"AB{k}", tag=f"AB{k}")
              for k in range(8)]
        u1T = psmall.tile([128, NT * 56], BF16, name="u1T", tag="u1T")
        p2T = psmall.tile([56, T], BF16, name="p2T", tag="p2T")
        vabd = psmall.tile([56, 512], BF16, name="vabd", tag="vabd")
        rec1 = psmall.tile([56, 1], F32, name="rec1", tag="rec1")

        # ---- vT: v (10 head slots incl halo) transposed, bf16, t-padded
        pvT = ctx.enter_context(tc.tile_pool(name="vT", bufs=1))
        vT = [pvT.tile([128, TPAD], BF16, name=f"vT{j}", tag=f"vT{j}")
              for j in range(5)]
        for j in range(5):
            nc.vector.memset(vT[j][:, 0:1], 0.0)
            nc.vector.memset(vT[j][:, TPAD - 1:TPAD], 0.0)

        # ---- xn: x natural layout, resident (for the agg matmuls) -----
        pxn = ctx.enter_context(tc.tile_pool(name="xnat", bufs=1))
        xnb = pxn.tile([128, NT * DIM], BF16, name="xnb", tag="xnb")
        for tt in range(NT):
            nc.gpsimd.dma_start(
                xnb[:, ts(tt, DIM)], xb[ts(tt, 128), :])

        def xn(tt):
            return xnb[:, ds(tt * DIM, DIM)]

        # weights for AB formation + agT
        pwsm = ctx.enter_context(tc.tile_pool(name="wsm", bufs=1))
        wqn_sb = pwsm.tile([128, 8, 512], BF16, name="wqn", tag="wqn")
        for kk in range(8):
            nc.scalar.dma_start(wqn_sb[:, kk, :], wqn[ts(kk, 128), :])
        wkt_sb = [pwsm.tile([128, DIM], BF16, name=f"wkt{j}", tag=f"wkt{j}")
                  for j in range(4)]
        wqt_sb = [pwsm.tile([128, DIM], BF16, name=f"wqt{j}", tag=f"wqt{j}")
                  for j in range(4)]
        for j in range(4):
            nc.scalar.dma_start(wkt_sb[j][:], wkt[ts(j, 128), :])
            nc.scalar.dma_start(wqt_sb[j][:], wqt[ts(j, 128), :])

        # ---- phase 1: xT load + v matmul + xsum reduces ---------------
        import contextlib as _cl
        xT_stack = _cl.ExitStack()
        pxT = xT_stack.enter_context(tc.tile_pool(name="xT", bufs=1))
        xTb = pxT.tile([128, 8 * T], BF16, name="xTb", tag="xTb")
        for kk in range(8):
            nc.sync.dma_start(
                xTb[:, ds(kk * T, T)], xbT[ts(kk, 128), :])

        def xT(kk):
            return xTb[:, ds(kk * T, T)]

        with tc.tile_pool(name="wstream", bufs=3) as pw, \
             tc.tile_pool(name="mmps", bufs=4, space="PSUM") as pmm:
            for cg in range(5):
                wt = pw.tile([128, 8, 128], BF16, name="w", tag="w")
                for kk in range(8):
                    nc.sync.dma_start(
                        wt[:, kk, :], wv10[ts(kk, 128), ts(cg, 128)])
                for ch in range(NCH):
                    pm = pmm.tile([128, 512], F32, name="mm", tag="mm")
                    for kk in range(8):
                        nc.tensor.matmul(
                            pm[:], wt[:, kk, :], xT(kk)[:, ds(ch * 512, 512)],
                            start=(kk == 0), stop=(kk == 7),
                        )
                    eng = nc.scalar if (cg * NCH + ch) % 2 else nc.vector
                    _copy(eng, vT[cg][:, ds(1 + ch * 512, 512)], pm[:])

        # xsum: per-chunk column sums of x (via free-dim reduce on xT)
        for kk in range(8):
            for ch in range(NCH):
                nc.vector.reduce_sum(
                    xsumT[kk][:, ch:ch + 1], xT(kk)[:, ds(ch * 512, 512)],
                    axis=AX.X)
            nc.scalar.copy(xsumB[kk][:], xsumT[kk][:])

        # ---- agent + A/B formation ------------------------------------
        with tc.tile_pool(name="agps", bufs=1, space="PSUM") as pagp, \
             tc.tile_pool(name="abps", bufs=2, space="PSUM") as pabp:
            agJ = [pagp.tile([128, P], F32, name=f"agJ{j}", tag=f"agJ{j}")
                   for j in range(4)]
            for j in range(4):
                for kk in range(8):
                    nc.tensor.matmul(
                        agJ[j][:], wqn_sb[:, kk, ds(j * 128, 128)],
                        xsumB[kk][:], start=(kk == 0), stop=(kk == 7))
            # R[j]: block-diagonal agent^T * SC/512, bf16 [128, 14]
            for j in range(4):
                nc.vector.memset(R[j][:], 0.0)
                nc.scalar.activation(
                    R[j][0:64, 0:P], agJ[j][0:64, :], AF.Copy,
                    scale=SC / 512.0)
                nc.scalar.activation(
                    R[j][64:128, P:2 * P], agJ[j][64:128, :], AF.Copy,
                    scale=SC / 512.0)
            # AB[kk] = [A^T | B] slice [128 m, 112]
            for kk in range(8):
                pab = pabp.tile([128, 112], F32, name="ab", tag="ab")
                for j in range(4):
                    nc.tensor.matmul(
                        pab[:, ts(j, 14)], wkt_sb[j][:, ts(kk, 128)],
                        R[j][:], start=True, stop=True,
                        skip_group_check=True)
                    nc.tensor.matmul(
                        pab[:, ds(56 + j * 14, 14)],
                        wqt_sb[j][:, ts(kk, 128)],
                        R[j][:], start=True, stop=True,
                        skip_group_check=True)
                eng = nc.scalar if kk % 2 else nc.vector
                _copy(eng, AB[kk][:], pab[:])

        # ---- scores sweep: u1 = exp(x@A^T), p2 = softmax(x@B) ---------
        with tc.tile_pool(name="sps", bufs=2, space="PSUM") as pps, \
             tc.tile_pool(name="trps", bufs=2, space="PSUM") as ptr, \
             tc.tile_pool(name="stmp", bufs=3) as pst:
            for tt in range(NT):
                ps = pps.tile([128, 112], F32, name="s", tag="s")
                for kk in range(8):
                    nc.tensor.matmul(
                        ps[:], xT(kk)[:, ts(tt, 128)], AB[kk][:],
                        start=(kk == 0), stop=(kk == 7))
                nc.scalar.activation(
                    u1T[:, ts(tt, 56)], ps[:, 0:56], AF.Exp)
                u2 = pst.tile([128, 56], F32, name="u2", tag="u2")
                nc.scalar.activation(u2[:], ps[:, 56:112], AF.Exp)
                rs = pst.tile([128, 8], F32, name="rs", tag="rs")
                nc.vector.reduce_sum(
                    rs[:], u2.rearrange("p (h q) -> p h q", q=P), axis=AX.X)
                nc.vector.reciprocal(rs[:], rs[:])
                p2f = pst.tile([128, 56], F32, name="p2f", tag="p2f")
                nc.vector.tensor_tensor(
                    out=p2f.rearrange("p (h q) -> p h q", q=P),
                    in0=u2.rearrange("p (h q) -> p h q", q=P),
                    in1=rs[:, :, None].broadcast_to([128, 8, P]),
                    op=mybir.AluOpType.mult,
                )
                ptt = ptr.tile([56, 128], F32, name="p2t", tag="p2t")
                nc.tensor.transpose(ptt[:], p2f[:], idf_sb[:])
                nc.any.tensor_copy(p2T[:, ts(tt, 128)], ptt[:])

        xT_stack.close()  # free xTb SBUF

        # vO (odd-aligned slot pairs) + taps, early: SBUF->SBUF partition
        # shift DMAs overlap the scores/agg sections.
        ptaps = ctx.enter_context(tc.tile_pool(name="taps", bufs=1))
        taps_sb = ptaps.tile([128, 9, 128], BF16, name="taps", tag="taps")
        for k9 in range(9):
            nc.sync.dma_start(taps_sb[:, k9, :], taps[k9])
        pvO = ctx.enter_context(tc.tile_pool(name="vO", bufs=1))
        vO = [pvO.tile([128, TPAD], BF16, name=f"vO{i}", tag=f"vO{i}")
              for i in range(4)]
        for i in range(4):
            nc.sync.dma_start(vO[i][0:64, :], vT[i][64:128, :])
            nc.sync.dma_start(vO[i][64:128, :], vT[i + 1][0:64, :])
        wv_sb = pvO.tile([128, 8, 512], BF16, name="wv", tag="wvl")
        for kk in range(8):
            nc.scalar.dma_start(wv_sb[:, kk, :], wvloc[ts(kk, 128), :])

        # ---- stage-1 aggregation: a1 = u1^T @ x, va = (a1 @ Wv)*rec ---
        with tc.tile_pool(name="aggtmp", bufs=1) as pag, \
             tc.tile_pool(name="a1ps", bufs=1, space="PSUM") as pa1p, \
             tc.tile_pool(name="csps", bufs=1, space="PSUM") as pcsp, \
             tc.tile_pool(name="vaps", bufs=1, space="PSUM") as pvap, \
             tc.tile_pool(name="a2tps", bufs=2, space="PSUM") as pa2t:
            pa1 = [pa1p.tile([56, DIM], F32, name=f"a1_{e}", tag=f"a1_{e}")
                   for e in range(2)]
            pcs = pcsp.tile([56, 1], F32, name="cs", tag="cs")
            for tt in range(NT):
                e = tt % 2
                for half in range(2):
                    nc.tensor.matmul(
                        pa1[e][:, ts(half, 512)], u1T[:, ts(tt, 56)],
                        xn(tt)[:, ts(half, 512)],
                        start=(tt == e), stop=(tt >= NT - 2))
                nc.tensor.matmul(pcs[:], u1T[:, ts(tt, 56)], ones_sb[:],
                                 start=(tt == 0), stop=(tt == NT - 1))
            nc.vector.reciprocal(rec1[:], pcs[:])
            a2 = pag.tile([56, DIM], BF16, name="a2", tag="a2")
            nc.scalar.copy(a2[:], pa1[0][:])
            nc.vector.tensor_add(a2[:], a2[:], pa1[1][:])
            pva = pvap.tile([56, 512], F32, name="va", tag="va")
            a2ts_all = pag.tile([128, 8, 56], BF16, name="a2ts", tag="a2ts")
            for kk in range(8):
                pt = pa2t.tile([128, 56], BF16, name="a2t", tag="a2t")
                nc.tensor.transpose(
                    pt[:], a2[:, ts(kk, 128)], idb_sb[0:56, 0:56])
                nc.any.tensor_copy(a2ts_all[:, kk, :], pt[:])
                nc.tensor.matmul(pva[:], a2ts_all[:, kk, :],
                                 wv_sb[:, kk, :],
                                 start=(kk == 0), stop=(kk == 7))
            van = pag.tile([56, 512], BF16, name="van", tag="van")
            nc.vector.tensor_scalar(
                out=van[:], in0=pva[:], scalar1=rec1[:],
                scalar2=None, op0=mybir.AluOpType.mult,
            )
            nc.vector.memset(vabd[:], 0.0)
            for h in range(HL):
                nc.sync.dma_start(
                    vabd[ds(P * h, P), ds(64 * h, 64)],
                    van[ds(P * h, P), ds(64 * h, 64)])

        # ---- dwc (depthwise 3x3) + attention output, fused in PSUM ---
        pY = ctx.enter_context(tc.tile_pool(name="Ypool", bufs=1))
        Y = []
        with tc.tile_pool(name="dwcps", bufs=3, space="PSUM") as pdw:
            for i in range(4):
                src_by_kh = (vT[i], vO[i], vT[i + 1])
                Yi = pY.tile([128, T], BF16, name=f"Y{i}", tag=f"Y{i}")
                Y.append(Yi)
                for tc7 in range(NCH):
                    off = 1 + tc7 * 512
                    pd = pdw.tile([128, 512], F32, name="dwc", tag="dwc")
                    n = 0
                    for kh in range(3):
                        for kt in range(3):
                            nc.tensor.matmul(
                                pd[:], taps_sb[:, kh * 3 + kt, :],
                                src_by_kh[kh][:, ds(off + kt - 1, 512)],
                                start=(n == 0), stop=False)
                            n += 1
                    # stage-2 attention output: 10th accumulation step
                    nc.tensor.matmul(
                        pd[:], vabd[:, ts(i, 128)], p2T[:, ts(tc7, 512)],
                        start=False, stop=True)
                    if tc7 % 2 == 0:
                        nc.scalar.activation(
                            Yi[:, ts(tc7, 512)], pd[:], AF.Identity,
                            bias=cb_sb[:, 0:1])
                    else:
                        nc.vector.tensor_scalar(
                            out=Yi[:, ts(tc7, 512)], in0=pd[:],
                            scalar1=cb_sb[:, 0:1], scalar2=None,
                            op0=mybir.AluOpType.add)

        # ---- output projection: out = Y^T @ Wo ------------------------
        with tc.tile_pool(name="ostage", bufs=2) as pos, \
             tc.tile_pool(name="ops", bufs=4, space="PSUM") as pop:
            wo_sb = [pvO.tile([128, DIM], BF16, name=f"wo{k}", tag=f"vO{k}")
                     for k in range(4)]
            for k in range(4):
                nc.scalar.dma_start(wo_sb[k][:], wo[ts(k, 128), :])
            for tt in range(NT):
                po = pop.tile([128, DIM], F32, name="o", tag="o")
                for half in range(2):
                    for k in range(4):
                        nc.tensor.matmul(
                            po[:, ts(half, 512)],
                            Y[k][:, ts(tt, 128)],
                            wo_sb[k][:, ts(half, 512)],
                            start=(k == 0), stop=(k == 3))
                osta = pos.tile([128, 512], F32, name="osta", tag="osta")
                ostb = pos.tile([128, 512], F32, name="ostb", tag="ostb")
                nc.vector.tensor_copy(osta[:], po[:, 0:512])
                nc.scalar.copy(ostb[:], po[:, 512:DIM])
                nc.scalar.dma_start(outp[ts(tt, 128), 0:512], osta[:])
                nc.sync.dma_start(outp[ts(tt, 128), 512:DIM], ostb[:])


_NC_CACHE = None


def _get_nc():
    global _NC_CACHE
    if _NC_CACHE is None:
        _NC_CACHE = build_nc()
    return _NC_CACHE


def _prep_core_inputs(x, W_qkv, W_o, conv_w):
    bf = ml_dtypes.bfloat16
    ins = []
    idf_np = np.eye(128, dtype=np.float32)
    idb_np = np.eye(128, dtype=bf)
    # taps[kh*3+kt] = kron(I2, diag(conv_w[:, 0, kh, kt]))
    taps_np = np.zeros((9, 128, 128), dtype=bf)
    cw = np.asarray(conv_w, np.float32)
    for kh in range(3):
        for kt in range(3):
            dg = np.diag(cw[:, 0, kh, kt]).astype(bf)
            taps_np[kh * 3 + kt, 0:64, 0:64] = dg
            taps_np[kh * 3 + kt, 64:128, 64:128] = dg
    for c in range(8):
        b, g = c // 2, c % 2
        wq = W_qkv[:, 512 * g:512 * g + 512]
        wk = W_qkv[:, 1024 + 512 * g:1024 + 512 * g + 512]
        wv10 = np.zeros((DIM, 640), np.float32)
        for s in range(10):
            h = 8 * g - 1 + s
            if 0 <= h < 16:
                wv10[:, 64 * s:64 * s + 64] = \
                    W_qkv[:, 2048 + 64 * h:2048 + 64 * h + 64]
        ins.append({
            "xbT": np.ascontiguousarray(x[b].T).astype(bf),
            "xb": np.ascontiguousarray(x[b]).astype(bf),
            "wv10": np.ascontiguousarray(wv10).astype(bf),
            "wvloc": np.ascontiguousarray(wv10[:, 64:576]).astype(bf),
            "wqn": np.ascontiguousarray(wq).astype(bf),
            "wkt": np.ascontiguousarray(wk.T).astype(bf),
            "wqt": np.ascontiguousarray(wq.T).astype(bf),
            "wo": np.ascontiguousarray(
                W_o[512 * g:512 * g + 512, :]).astype(bf),
            "taps": taps_np,
            "convb2": np.zeros((128, 1), np.float32),
            "idf": idf_np,
            "idb": idb_np,
        })
    return ins


def kernel(x, W_qkv, W_o, b_o, conv_w, conv_b, _run_kwargs=None):
    x = np.asarray(x, np.float32)
    W_qkv = np.asarray(W_qkv, np.float32)
    W_o = np.asarray(W_o, np.float32)
    b_o = np.asarray(b_o, np.float32)
    conv_w = np.asarray(conv_w, np.float32)
    conv_b = np.asarray(conv_b, np.float32)

    ins = _prep_core_inputs(x, W_qkv, W_o, conv_w)
    cb2 = np.tile(conv_b, 2).astype(np.float32).reshape(128, 1)
    for m in ins:
        m["convb2"] = cb2

    nc = _get_nc()
    res = bass_utils.run_bass_kernel_spmd(
        nc, ins, core_ids=list(range(8)), **(_run_kwargs or {}))
    outs = [r["outp"] for r in res.results]
    B = x.shape[0]
    full = np.empty((B, T, DIM), np.float32)
    for b in range(B):
        full[b] = outs[2 * b] + outs[2 * b + 1] + b_o[None, :]
    if _run_kwargs:
        kernel.last_results = res
    return full


# revision 13
# speedup vs baseline: 1.7274x; 1.3588x over previous
"""AgentAttention TRN2 kernel: 8 cores = 4 batches x 2 head-groups.

Reference computation (B=4, T=3584, dim=1024, H=16, D=64, P=7):
  qkv = x @ W_qkv -> q,k,v [B,H,T,D]
  agent = avgpool_T(q) [B,H,P,D]
  v_agent = softmax(agent*SC @ k^T) @ v
  out_att = softmax(q*SC @ agent^T) @ v_agent
  dwc = depthwise3x3 over (H,T) of v
  out = (out_att + dwc) 'b h t d -> b t (h d)' @ W_o + b_o

Core c handles batch c//2, heads [8g, 8g+8) with g=c%2. The two partial
outputs per batch are summed on the host (+ b_o).

Structure (all engine-time figures per the TRN2 cost model):
 - q and k are never materialized. Scores contract against x directly:
     s1^T = x @ A^T,  A = (agent*SC) @ Wk^T;   s2 = x @ B,  B = Wq @ agent^T*SC
   agent = (pooled x) @ Wq, with the pooling done as free-dim reduces of
   x^T (host supplies x^T). The only large qkv-side matmul is v
   (10 head slots incl conv halo, 640 cols).
 - Stage-1 aggregation re-associated through x: va = (u1^T @ x) @ Wv,
   emitted transposed (a1^T accumulated in one PSUM bank, 8 col-groups)
   and interleaved tile-by-tile into the scores sweep.
 - The stage-2 attention output matmul accumulates into the depthwise
   conv PSUM tile as a 10th accumulation step.
 - DMA transfers and HWDGE generation are each globally serialized in
   the cost model, so: weights are packed host-side into one blob DMA,
   x^T arrives as 32 column-slab DMAs ordered so PE can start after the
   first slab, x-natural streams through a rolling pool during the
   scores sweep, and the output is written bf16, one DMA per token tile
   on the software-DGE (Pool) path to keep HWDGE free.
"""

import numpy as np
import ml_dtypes

import concourse.bass as bass
import concourse.bacc as bacc
import concourse.mybir as mybir
import concourse.tile as tile
from concourse.bass import ts, ds
from concourse import bass_utils

F32 = mybir.dt.float32
BF16 = mybir.dt.bfloat16
AX = mybir.AxisListType
AF = mybir.ActivationFunctionType
AL = mybir.AluOpType

T, DIM, D, P = 3584, 1024, 64, 7
HL = 8                      # local heads per core
SC = D ** -0.5
NT = T // 128               # 28 token tiles of 128
NCH = T // 512              # 7 chunks of 512
TPAD = T + 2                # vT padded with one zero col each side

# blob layout (bf16, per-partition column offsets)
OWQN, OWKT, OWQT, OWOP, OTAPS = 0, 4096, 8192, 12288, 16384
BLOBW = OTAPS + 9 * 128     # 17536


def build_nc(skip=()):
    # Bacc (not plain Bass): its compile() runs generate_event_semaphores,
    # which splits multi-wait sync_info into InstEventSemaphore -- TRN2
    # instructions can carry at most one embedded wait.
    nc = bacc.Bacc("TRN2", target_bir_lowering=False)

    xbT = nc.dram_tensor("xbT", [DIM, T], BF16, kind="ExternalInput")
    xb = nc.dram_tensor("xb", [T, DIM], BF16, kind="ExternalInput")
    wvp = nc.dram_tensor("wvp", [128, 8, 640], BF16, kind="ExternalInput")
    blob = nc.dram_tensor("blob", [128, BLOBW], BF16, kind="ExternalInput")
    fblob = nc.dram_tensor("fblob", [128, 129], F32, kind="ExternalInput")
    outp = nc.dram_tensor("outp", [T, DIM], BF16, kind="ExternalOutput")

    with tile.TileContext(nc) as tc:
        _emit(nc, tc, xbT, xb, wvp, blob, fblob, outp)
    nc.compile()
    return nc


def _copy(eng, out, in_):
    if hasattr(eng, "activation"):
        eng.copy(out, in_)
    else:
        eng.tensor_copy(out, in_)


def _emit(nc, tc, xbT, xb, wvp, blob, fblob, outp):
    import contextlib
    ctx = contextlib.ExitStack()
    with ctx:
        # ---- constants + weight blob ----------------------------------
        pconst = ctx.enter_context(tc.tile_pool(name="const", bufs=1))
        fb_sb = pconst.tile([128, 129], F32, name="fb", tag="fb")
        nc.sync.dma_start(fb_sb[:], fblob[:])
        idf_sb = fb_sb[:, 0:128]
        cb_sb = fb_sb[:, 128:129]
        ones_sb = pconst.tile([128, 1], BF16, name="ones", tag="ones")
        nc.vector.memset(ones_sb[:], 1.0)

        pwvp = ctx.enter_context(tc.tile_pool(name="wvp", bufs=1))
        wvp_sb = pwvp.tile([128, 8, 640], BF16, name="wvp", tag="wvp")
        nc.sync.dma_start(wvp_sb[:], wvp[:])

        psmall = ctx.enter_context(tc.tile_pool(name="small", bufs=1))
        xsumT = [psmall.tile([128, P], F32, name=f"xsT{k}", tag=f"xsT{k}")
                 for k in range(8)]
        xsumB = [psmall.tile([128, P], BF16, name=f"xsB{k}", tag=f"xsB{k}")
                 for k in range(8)]
        R = [psmall.tile([128, 2 * P], BF16, name=f"R{j}", tag=f"R{j}")
             for j in range(4)]
        AB = [psmall.tile([128, 112], BF16, name=f"AB{k}", tag=f"AB{k}")
              for k in range(8)]
        u1T = psmall.tile([128, NT * 56], BF16, name="u1T", tag="u1T")
        p2T = psmall.tile([56, T], BF16, name="p2T", tag="p2T")
        a1sb = psmall.tile([128, 8 * 56], BF16, name="a1sb", tag="a1sb")
        vabd = psmall.tile([56, 512], BF16, name="vabd", tag="vabd")
        rec1 = psmall.tile([56, 1], F32, name="rec1", tag="rec1")

        # ---- vT: v (10 head slots incl halo) transposed, bf16, t-padded
        pvT = ctx.enter_context(tc.tile_pool(name="vT", bufs=1))
        vT = [pvT.tile([128, TPAD], BF16, name=f"vT{j}", tag=f"vT{j}")
              for j in range(5)]
        for j in range(5):
            nc.vector.memset(vT[j][:, 0:1], 0.0)
            nc.vector.memset(vT[j][:, TPAD - 1:TPAD], 0.0)

        # pools that outlive xT must be created before it (LIFO release)
        pblob = ctx.enter_context(tc.tile_pool(name="blob", bufs=1))
        pvO = ctx.enter_context(tc.tile_pool(name="vO", bufs=1))
        pxn = ctx.enter_context(tc.tile_pool(name="xnat", bufs=8))

        # ---- phase 1: xT slab loads + v matmul ------------------------
        # xT arrives in 4 column slabs x 8 kk tiles; v-matmul chains are
        # ordered by slab so PE starts once slab 0 lands.
        import contextlib as _cl
        xT_stack = _cl.ExitStack()
        pxT = xT_stack.enter_context(tc.tile_pool(name="xT", bufs=1))
        xTb = pxT.tile([128, 8 * T], BF16, name="xTb", tag="xTb")

        def xT(kk):
            return xTb[:, ds(kk * T, T)]

        SLABS = [(0, 1024, [0, 1]), (1024, 1024, [2, 3]),
                 (2048, 1024, [4, 5]), (3072, 512, [6])]
        slab_chunks = []
        for off, w, chunks in SLABS:
            for kk in range(8):
                nc.sync.dma_start(
                    xTb[:, ds(kk * T + off, w)],
                    xbT[ts(kk, 128), ds(off, w)])
            slab_chunks.append(chunks)

        with tc.tile_pool(name="mmps", bufs=4, space="PSUM") as pmm:
            nmm = 0
            for chunks in slab_chunks:
                for cg in range(5):
                    for ch in chunks:
                        pm = pmm.tile([128, 512], F32, name="mm", tag="mm")
                        for kk in range(8):
                            nc.tensor.matmul(
                                pm[:], wvp_sb[:, kk, ds(cg * 128, 128)],
                                xT(kk)[:, ds(ch * 512, 512)],
                                start=(kk == 0), stop=(kk == 7),
                            )
                        eng = nc.scalar if nmm % 2 else nc.vector
                        nmm += 1
                        _copy(eng, vT[cg][:, ds(1 + ch * 512, 512)], pm[:])

        # weight blob (queued on sync after the xT slabs)
        blob_sb = pblob.tile([128, BLOBW], BF16, name="blob", tag="blob")
        nc.sync.dma_start(blob_sb[:], blob[:])
        wqn_v = blob_sb[:, OWQN:OWQN + 4096].rearrange(
            "p (k c) -> p k c", c=512)
        wkt_v = blob_sb[:, OWKT:OWKT + 4096].rearrange(
            "p (k c) -> p k c", c=1024)
        wqt_v = blob_sb[:, OWQT:OWQT + 4096].rearrange(
            "p (k c) -> p k c", c=1024)
        wop_v = blob_sb[:, OWOP:OWOP + 4096].rearrange(
            "p (k c) -> p k c", c=1024)
        taps_v = blob_sb[:, OTAPS:OTAPS + 1152].rearrange(
            "p (k c) -> p k c", c=128)

        # xsum: per-chunk column sums of x (free-dim reduce on xT)
        for kk in range(8):
            for ch in range(NCH):
                nc.vector.reduce_sum(
                    xsumT[kk][:, ch:ch + 1], xT(kk)[:, ds(ch * 512, 512)],
                    axis=AX.X)
            nc.scalar.copy(xsumB[kk][:], xsumT[kk][:])

        # ---- agent + A/B formation ------------------------------------
        with tc.tile_pool(name="agps", bufs=1, space="PSUM") as pagp, \
             tc.tile_pool(name="abps", bufs=2, space="PSUM") as pabp:
            agJ = [pagp.tile([128, P], F32, name=f"agJ{j}", tag=f"agJ{j}")
                   for j in range(4)]
            for j in range(4):
                for kk in range(8):
                    nc.tensor.matmul(
                        agJ[j][:], wqn_v[:, kk, ds(j * 128, 128)],
                        xsumB[kk][:], start=(kk == 0), stop=(kk == 7))
            # R[j]: block-diagonal agent^T * SC/512, bf16 [128, 14]
            for j in range(4):
                nc.vector.memset(R[j][:], 0.0)
                nc.scalar.activation(
                    R[j][0:64, 0:P], agJ[j][0:64, :], AF.Copy,
                    scale=SC / 512.0)
                nc.scalar.activation(
                    R[j][64:128, P:2 * P], agJ[j][64:128, :], AF.Copy,
                    scale=SC / 512.0)
            # AB[kk] = [A^T | B] slice [128 m, 112]
            for kk in range(8):
                pab = pabp.tile([128, 112], F32, name="ab", tag="ab")
                for j in range(4):
                    nc.tensor.matmul(
                        pab[:, ts(j, 14)], wkt_v[:, j, ts(kk, 128)],
                        R[j][:], start=True, stop=True,
                        skip_group_check=True)
                    nc.tensor.matmul(
                        pab[:, ds(56 + j * 14, 14)],
                        wqt_v[:, j, ts(kk, 128)],
                        R[j][:], start=True, stop=True,
                        skip_group_check=True)
                eng = nc.scalar if kk % 2 else nc.vector
                _copy(eng, AB[kk][:], pab[:])

        # vO (odd-aligned slot pairs): SBUF->SBUF partition-shift DMAs,
        # overlapping the scores sweep (only needed by dwc).
        vO = [pvO.tile([128, TPAD], BF16, name=f"vO{i}", tag=f"vO{i}")
              for i in range(4)]
        for i in range(4):
            nc.scalar.dma_start(vO[i][0:64, :], vT[i][64:128, :])
            nc.scalar.dma_start(vO[i][64:128, :], vT[i + 1][0:64, :])

        # ---- scores sweep + interleaved stage-1 aggregation -----------
        # Per tile tt: scores matmuls + exp/softmax; the agg matmuls and
        # p2 transpose for tile tt-1 are emitted one iteration later so
        # PE never waits on the exp/softmax chain.
        xn_tiles = {}

        def load_xn(tt):
            xn = pxn.tile([128, DIM], BF16, name="xn", tag="xn")
            nc.sync.dma_start(xn[:], xb[ts(tt, 128), :])
            xn_tiles[tt] = xn

        for tt in range(6):
            load_xn(tt)

        with tc.tile_pool(name="sps", bufs=2, space="PSUM") as pps, \
             tc.tile_pool(name="trps", bufs=2, space="PSUM") as ptr, \
             tc.tile_pool(name="a1ps", bufs=1, space="PSUM") as pa1p, \
             tc.tile_pool(name="csps", bufs=1, space="PSUM") as pcsp, \
             tc.tile_pool(name="stmp", bufs=3) as pst:
            a1T = pa1p.tile([128, 8 * 56], F32, name="a1T", tag="a1T")
            pcs = pcsp.tile([56, 1], F32, name="cs", tag="cs")
            prev_p2f = None

            def emit_agg(tt):
                for kk in range(8):
                    nc.tensor.matmul(
                        a1T[:, ts(kk, 56)],
                        xn_tiles[tt][:, ts(kk, 128)], u1T[:, ts(tt, 56)],
                        start=(tt == 0), stop=(tt == NT - 1),
                        skip_group_check=True)
                del xn_tiles[tt]
                nc.tensor.matmul(pcs[:], u1T[:, ts(tt, 56)], ones_sb[:],
                                 start=(tt == 0), stop=(tt == NT - 1))

            def emit_p2t(tt, p2f):
                ptt = ptr.tile([56, 128], F32, name="p2t", tag="p2t")
                nc.tensor.transpose(ptt[:], p2f[:], idf_sb)
                nc.any.tensor_copy(p2T[:, ts(tt, 128)], ptt[:])

            for tt in range(NT + 1):
                if tt < NT:
                    if tt + 6 < NT:
                        load_xn(tt + 6)
                    ps = pps.tile([128, 112], F32, name="s", tag="s")
                    for kk in range(8):
                        nc.tensor.matmul(
                            ps[:], xT(kk)[:, ts(tt, 128)], AB[kk][:],
                            start=(kk == 0), stop=(kk == 7))
                if tt > 0:
                    emit_agg(tt - 1)
                    emit_p2t(tt - 1, prev_p2f)
                if tt < NT:
                    nc.scalar.activation(
                        u1T[:, ts(tt, 56)], ps[:, 0:56], AF.Exp)
                    u2 = pst.tile([128, 56], F32, name="u2", tag="u2")
                    nc.scalar.activation(u2[:], ps[:, 56:112], AF.Exp)
                    rs = pst.tile([128, 8], F32, name="rs", tag="rs")
                    nc.vector.reduce_sum(
                        rs[:], u2.rearrange("p (h q) -> p h q", q=P),
                        axis=AX.X)
                    nc.vector.reciprocal(rs[:], rs[:])
                    p2f = pst.tile([128, 56], F32, name="p2f", tag="p2f")
                    nc.vector.tensor_tensor(
                        out=p2f.rearrange("p (h q) -> p h q", q=P),
                        in0=u2.rearrange("p (h q) -> p h q", q=P),
                        in1=rs[:, :, None].broadcast_to([128, 8, P]),
                        op=AL.mult,
                    )
                    prev_p2f = p2f

            nc.vector.reciprocal(rec1[:], pcs[:])
            eng = nc.scalar
            eng.copy(a1sb[:], a1T[:])

        xT_stack.close()  # free xTb SBUF

        # ---- v_agent: va = (a1 @ Wv) * rec, diag blocks -> vabd -------
        with tc.tile_pool(name="vaps", bufs=1, space="PSUM") as pvap, \
             tc.tile_pool(name="vtmp", bufs=1) as pvt:
            pva = pvap.tile([56, 512], F32, name="va", tag="va")
            for kk in range(8):
                nc.tensor.matmul(pva[:], a1sb[:, ts(kk, 56)],
                                 wvp_sb[:, kk, ds(64, 512)],
                                 start=(kk == 0), stop=(kk == 7))
            van = pvt.tile([56, 512], BF16, name="van", tag="van")
            nc.vector.tensor_scalar(
                out=van[:], in0=pva[:], scalar1=rec1[:],
                scalar2=None, op0=AL.mult,
            )
            nc.vector.memset(vabd[:], 0.0)
            for h in range(HL):
                nc.sync.dma_start(
                    vabd[ds(P * h, P), ds(64 * h, 64)],
                    van[ds(P * h, P), ds(64 * h, 64)])

        # ---- dwc (depthwise 3x3) + attention output, fused in PSUM ---
        pY = ctx.enter_context(tc.tile_pool(name="Ypool", bufs=1))
        Y = []
        with tc.tile_pool(name="dwcps", bufs=3, space="PSUM") as pdw:
            for i in range(4):
                src_by_kh = (vT[i], vO[i], vT[i + 1])
                Yi = pY.tile([128, T], BF16, name=f"Y{i}", tag=f"Y{i}")
                Y.append(Yi)
                for tc7 in range(NCH):
                    off = 1 + tc7 * 512
                    pd = pdw.tile([128, 512], F32, name="dwc", tag="dwc")
                    n = 0
                    for kh in range(3):
                        for kt in range(3):
                            nc.tensor.matmul(
                                pd[:], taps_v[:, kh * 3 + kt, :],
                                src_by_kh[kh][:, ds(off + kt - 1, 512)],
                                start=(n == 0), stop=False)
                            n += 1
                    # stage-2 attention output: 10th accumulation step
                    nc.tensor.matmul(
                        pd[:], vabd[:, ts(i, 128)], p2T[:, ts(tc7, 512)],
                        start=False, stop=True)
                    if tc7 % 2 == 0:
                        nc.scalar.activation(
                            Yi[:, ts(tc7, 512)], pd[:], AF.Identity,
                            bias=cb_sb)
                    else:
                        nc.vector.tensor_scalar(
                            out=Yi[:, ts(tc7, 512)], in0=pd[:],
                            scalar1=cb_sb, scalar2=None,
                            op0=AL.add)

        # ---- output projection: out = Y^T @ Wo ------------------------
        with tc.tile_pool(name="ostage", bufs=3) as pos, \
             tc.tile_pool(name="ops", bufs=4, space="PSUM") as pop:
            for tt in range(NT):
                po = pop.tile([128, DIM], F32, name="o", tag="o")
                for half in range(2):
                    for k in range(4):
                        nc.tensor.matmul(
                            po[:, ts(half, 512)],
                            Y[k][:, ts(tt, 128)],
                            wop_v[:, k, ts(half, 512)],
                            start=(k == 0), stop=(k == 3))
                ost = pos.tile([128, DIM], BF16, name="ost", tag="ost")
                _copy(nc.scalar if tt % 2 else nc.vector, ost[:], po[:])
                nc.gpsimd.dma_start(outp[ts(tt, 128), :], ost[:])


_NC_CACHE = None


def _get_nc():
    global _NC_CACHE
    if _NC_CACHE is None:
        _NC_CACHE = build_nc()
    return _NC_CACHE


def _prep_core_inputs(x, W_qkv, W_o, conv_w, conv_b):
    bf = ml_dtypes.bfloat16
    ins = []
    # taps[kh*3+kt] = kron(I2, diag(conv_w[:, 0, kh, kt]))
    taps_np = np.zeros((9, 128, 128), dtype=np.float32)
    cw = np.asarray(conv_w, np.float32)
    for kh in range(3):
        for kt in range(3):
            dg = np.diag(cw[:, 0, kh, kt])
            taps_np[kh * 3 + kt, 0:64, 0:64] = dg
            taps_np[kh * 3 + kt, 64:128, 64:128] = dg
    fblob = np.zeros((128, 129), np.float32)
    fblob[:, 0:128] = np.eye(128, dtype=np.float32)
    fblob[:, 128] = np.tile(conv_b, 2)

    def pack(w, k):  # [k*128, c] -> [128, k, c]
        c = w.shape[1]
        return np.ascontiguousarray(
            w.reshape(k, 128, c).transpose(1, 0, 2))

    for c in range(8):
        b, g = c // 2, c % 2
        wq = W_qkv[:, 512 * g:512 * g + 512]
        wk = W_qkv[:, 1024 + 512 * g:1024 + 512 * g + 512]
        wv10 = np.zeros((DIM, 640), np.float32)
        for s in range(10):
            h = 8 * g - 1 + s
            if 0 <= h < 16:
                wv10[:, 64 * s:64 * s + 64] = \
                    W_qkv[:, 2048 + 64 * h:2048 + 64 * h + 64]
        blob = np.empty((128, BLOBW), np.float32)
        blob[:, OWQN:OWQN + 4096] = pack(wq, 8).reshape(128, 4096)
        blob[:, OWKT:OWKT + 4096] = pack(
            np.ascontiguousarray(wk.T), 4).reshape(128, 4096)
        blob[:, OWQT:OWQT + 4096] = pack(
            np.ascontiguousarray(wq.T), 4).reshape(128, 4096)
        blob[:, OWOP:OWOP + 4096] = pack(
            np.ascontiguousarray(W_o[512 * g:512 * g + 512, :]),
            4).reshape(128, 4096)
        blob[:, OTAPS:OTAPS + 1152] = taps_np.transpose(1, 0, 2).reshape(
            128, 1152)
        ins.append({
            "xbT": np.ascontiguousarray(x[b].T).astype(bf),
            "xb": np.ascontiguousarray(x[b]).astype(bf),
            "wvp": pack(wv10, 8).astype(bf),
            "blob": blob.astype(bf),
            "fblob": fblob,
        })
    return ins


def kernel(x, W_qkv, W_o, b_o, conv_w, conv_b, _run_kwargs=None):
    x = np.asarray(x, np.float32)
    W_qkv = np.asarray(W_qkv, np.float32)
    W_o = np.asarray(W_o, np.float32)
    b_o = np.asarray(b_o, np.float32)
    conv_w = np.asarray(conv_w, np.float32)
    conv_b = np.asarray(conv_b, np.float32)

    ins = _prep_core_inputs(x, W_qkv, W_o, conv_w, conv_b)

    nc = _get_nc()
    res = bass_utils.run_bass_kernel_spmd(
        nc, ins, core_ids=list(range(8)), **(_run_kwargs or {}))
    outs = [r["outp"] for r in res.results]
    B = x.shape[0]
    full = np.empty((B, T, DIM), np.float32)
    for b in range(B):
        full[b] = (outs[2 * b].astype(np.float32)
                   + outs[2 * b + 1].astype(np.float32) + b_o[None, :])
    if _run_kwargs:
        kernel.last_results = res
    return full


# revision 25
# speedup vs baseline: 1.7954x; 1.0393x over previous
"""AgentAttention TRN2 kernel: 8 cores = 4 batches x 2 head-groups.

Reference computation (B=4, T=3584, dim=1024, H=16, D=64, P=7):
  qkv = x @ W_qkv -> q,k,v [B,H,T,D]
  agent = avgpool_T(q) [B,H,P,D]
  v_agent = softmax(agent*SC @ k^T) @ v
  out_att = softmax(q*SC @ agent^T) @ v_agent
  dwc = depthwise3x3 over (H,T) of v
  out = (out_att + dwc) 'b h t d -> b t (h d)' @ W_o + b_o

Core c handles batch c//2, heads [8g, 8g+8) with g=c%2. The two partial
outputs per batch are summed on the host (+ b_o).

Structure (all engine-time figures per the TRN2 cost model):
 - q and k are never materialized. Scores contract against x directly:
     s1^T = x @ A^T,  A = (agent*SC) @ Wk^T;   s2 = x @ B,  B = Wq @ agent^T*SC
   agent = (pooled x) @ Wq, with the pooling done as free-dim reduces of
   x^T (host supplies x^T). The only large qkv-side matmul is v
   (10 head slots incl conv halo, 640 cols).
 - Stage-1 aggregation re-associated through x: va = (u1^T @ x) @ Wv,
   emitted transposed (a1^T accumulated in one PSUM bank, 8 col-groups)
   and interleaved tile-by-tile into the scores sweep.
 - The stage-2 attention output matmul accumulates into the depthwise
   conv PSUM tile as a 10th accumulation step.
 - DMA transfers and HWDGE generation are each globally serialized in
   the cost model, so: weights are packed host-side into one blob DMA,
   x^T arrives as 32 column-slab DMAs ordered so PE can start after the
   first slab, x-natural streams through a rolling pool during the
   scores sweep, and the output is written bf16, one DMA per token tile
   on the software-DGE (Pool) path to keep HWDGE free.
"""

import numpy as np
import ml_dtypes

import concourse.bass as bass
import concourse.bacc as bacc
import concourse.mybir as mybir
import concourse.tile as tile
from concourse.bass import ts, ds
from concourse import bass_utils

F32 = mybir.dt.float32
BF16 = mybir.dt.bfloat16
AX = mybir.AxisListType
AF = mybir.ActivationFunctionType
AL = mybir.AluOpType

T, DIM, D, P = 3584, 1024, 64, 7
HL = 8                      # local heads per core
SC = D ** -0.5
NT = T // 128               # 28 token tiles of 128
NCH = T // 512              # 7 chunks of 512
TPAD = T + 2                # vT padded with one zero col each side

# blob layout (bf16, per-partition column offsets)
OWQN, OWKT, OWQT, OWOP, OTAPS = 0, 4096, 8192, 12288, 16384
OCBT = OTAPS + 9 * 128      # conv_b tiled row, replicated on all partitions
BLOBW = OCBT + 512          # 18048


def build_nc(skip=()):
    # Bacc (not plain Bass): its compile() runs generate_event_semaphores,
    # which splits multi-wait sync_info into InstEventSemaphore -- TRN2
    # instructions can carry at most one embedded wait.
    nc = bacc.Bacc("TRN2", target_bir_lowering=False)

    xbT = nc.dram_tensor("xbT", [DIM, T], BF16, kind="ExternalInput")
    xb = nc.dram_tensor("xb", [T, DIM], BF16, kind="ExternalInput")
    wvpa = nc.dram_tensor("wvpa", [128, 1024], BF16, kind="ExternalInput")
    wvpb = nc.dram_tensor("wvpb", [128, 8, 512], BF16, kind="ExternalInput")
    blob = nc.dram_tensor("blob", [128, BLOBW], BF16, kind="ExternalInput")
    fblob = nc.dram_tensor("fblob", [128, 138], F32, kind="ExternalInput")
    outp = nc.dram_tensor("outp", [T, DIM], BF16, kind="ExternalOutput")

    with tile.TileContext(nc) as tc:
        _emit(nc, tc, xbT, xb, wvpa, wvpb, blob, fblob, outp)
    nc.compile()
    return nc


def _copy(eng, out, in_):
    if hasattr(eng, "activation"):
        eng.copy(out, in_)
    else:
        eng.tensor_copy(out, in_)


def _emit(nc, tc, xbT, xb, wvpa, wvpb, blob, fblob, outp):
    import contextlib
    ctx = contextlib.ExitStack()
    with ctx:
        # ---- constants + weight blob ----------------------------------
        pconst = ctx.enter_context(tc.tile_pool(name="const", bufs=1))
        fb_sb = pconst.tile([128, 138], F32, name="fb", tag="fb")
        nc.sync.dma_start(fb_sb[:], fblob[:])
        idf_sb = fb_sb[:, 0:128]
        wcol = fb_sb[:, 129:138]    # per-partition dwc tap weights
        ones_sb = pconst.tile([128, 1], BF16, name="ones", tag="ones")
        nc.vector.memset(ones_sb[:], 1.0)

        pwvp = ctx.enter_context(tc.tile_pool(name="wvp", bufs=1))
        wvpa_sb = pwvp.tile([128, 1024], BF16, name="wvpa", tag="wvpa")
        nc.sync.dma_start(wvpa_sb[:], wvpa[:])
        wvpa_v = wvpa_sb.rearrange("p (k c) -> p k c", c=128)
        wvpb_sb = pwvp.tile([128, 8, 512], BF16, name="wvpb", tag="wvpb")

        def wv_lhs(kk, cg):
            if cg == 0:
                return wvpa_v[:, kk, :]
            return wvpb_sb[:, kk, ds((cg - 1) * 128, 128)]

        psmall = ctx.enter_context(tc.tile_pool(name="small", bufs=1))
        xsumT = [psmall.tile([128, P], F32, name=f"xsT{k}", tag=f"xsT{k}")
                 for k in range(8)]
        xsumB = [psmall.tile([128, P], BF16, name=f"xsB{k}", tag=f"xsB{k}")
                 for k in range(8)]
        R = [psmall.tile([128, 2 * P], BF16, name=f"R{j}", tag=f"R{j}")
             for j in range(4)]
        AB = [psmall.tile([128, 112], BF16, name=f"AB{k}", tag=f"AB{k}")
              for k in range(8)]
        u1T = psmall.tile([128, NT * 56], BF16, name="u1T", tag="u1T")
        p2T = psmall.tile([56, T], BF16, name="p2T", tag="p2T")
        a1sb = psmall.tile([128, 8 * 56], BF16, name="a1sb", tag="a1sb")
        vabd = psmall.tile([56, 512], BF16, name="vabd", tag="vabd")
        rec1 = psmall.tile([56, 1], F32, name="rec1", tag="rec1")

        # ---- vT: v (10 head slots incl halo) transposed, bf16, t-padded
        pvT = ctx.enter_context(tc.tile_pool(name="vT", bufs=1))
        vT = [pvT.tile([128, TPAD], BF16, name=f"vT{j}", tag=f"vT{j}")
              for j in range(5)]
        for j in range(5):
            nc.vector.memset(vT[j][:, 0:1], 0.0)
            nc.vector.memset(vT[j][:, TPAD - 1:TPAD], 0.0)

        # pools that outlive xT must be created before it (LIFO release)
        pblob = ctx.enter_context(tc.tile_pool(name="blob", bufs=1))
        pvO = ctx.enter_context(tc.tile_pool(name="vO", bufs=1))
        pxn = ctx.enter_context(tc.tile_pool(name="xnat", bufs=8))

        # ---- phase 1: xT slab loads + v matmul ------------------------
        # xT arrives in 4 column slabs x 8 kk tiles; v-matmul chains are
        # ordered by slab so PE starts once slab 0 lands.
        import contextlib as _cl
        xT_stack = _cl.ExitStack()
        pxT = xT_stack.enter_context(tc.tile_pool(name="xT", bufs=1))
        xTb = pxT.tile([128, 8 * T], BF16, name="xTb", tag="xTb")

        def xT(kk):
            return xTb[:, ds(kk * T, T)]

        SLABS = [(0, 1024, [0, 1]), (1024, 1024, [2, 3]),
                 (2048, 1024, [4, 5]), (3072, 512, [6])]
        slab_chunks = []
        for si, (off, w, chunks) in enumerate(SLABS):
            for kk in range(8):
                nc.sync.dma_start(
                    xTb[:, ds(kk * T + off, w)],
                    xbT[ts(kk, 128), ds(off, w)])
            if si == 0:
                nc.sync.dma_start(wvpb_sb[:], wvpb[:])
            slab_chunks.append(chunks)

        with tc.tile_pool(name="mmps", bufs=4, space="PSUM") as pmm:
            nmm = 0
            for chunks in slab_chunks:
                for cg in range(5):
                    for ch in chunks:
                        pm = pmm.tile([128, 512], F32, name="mm", tag="mm")
                        for kk in range(8):
                            nc.tensor.matmul(
                                pm[:], wv_lhs(kk, cg),
                                xT(kk)[:, ds(ch * 512, 512)],
                                start=(kk == 0), stop=(kk == 7),
                            )
                        eng = nc.scalar if nmm % 2 else nc.vector
                        nmm += 1
                        _copy(eng, vT[cg][:, ds(1 + ch * 512, 512)], pm[:])

        # weight blob (queued on sync after the xT slabs)
        blob_sb = pblob.tile([128, BLOBW], BF16, name="blob", tag="blob")
        nc.sync.dma_start(blob_sb[:], blob[:])
        wqn_v = blob_sb[:, OWQN:OWQN + 4096].rearrange(
            "p (k c) -> p k c", c=512)
        wkt_v = blob_sb[:, OWKT:OWKT + 4096].rearrange(
            "p (k c) -> p k c", c=1024)
        wqt_v = blob_sb[:, OWQT:OWQT + 4096].rearrange(
            "p (k c) -> p k c", c=1024)
        wop_v = blob_sb[:, OWOP:OWOP + 4096].rearrange(
            "p (k c) -> p k c", c=1024)
        taps_v = blob_sb[:, OTAPS:OTAPS + 1152].rearrange(
            "p (k c) -> p k c", c=128)

        # xsum: per-chunk column sums of x (free-dim reduce on xT)
        for kk in range(8):
            for ch in range(NCH):
                nc.vector.reduce_sum(
                    xsumT[kk][:, ch:ch + 1], xT(kk)[:, ds(ch * 512, 512)],
                    axis=AX.X)
            nc.scalar.copy(xsumB[kk][:], xsumT[kk][:])

        # ---- agent + A/B formation ------------------------------------
        with tc.tile_pool(name="agps", bufs=1, space="PSUM") as pagp, \
             tc.tile_pool(name="abps", bufs=2, space="PSUM") as pabp:
            agJ = [pagp.tile([128, P], F32, name=f"agJ{j}", tag=f"agJ{j}")
                   for j in range(4)]
            for j in range(4):
                for kk in range(8):
                    nc.tensor.matmul(
                        agJ[j][:], wqn_v[:, kk, ds(j * 128, 128)],
                        xsumB[kk][:], start=(kk == 0), stop=(kk == 7))
            # R[j]: block-diagonal agent^T * SC/512, bf16 [128, 14]
            for j in range(4):
                nc.vector.memset(R[j][:], 0.0)
                nc.scalar.activation(
                    R[j][0:64, 0:P], agJ[j][0:64, :], AF.Copy,
                    scale=SC / 512.0)
                nc.scalar.activation(
                    R[j][64:128, P:2 * P], agJ[j][64:128, :], AF.Copy,
                    scale=SC / 512.0)
            # AB[kk] = [A^T | B] slice [128 m, 112]
            for kk in range(8):
                pab = pabp.tile([128, 112], F32, name="ab", tag="ab")
                for j in range(4):
                    nc.tensor.matmul(
                        pab[:, ts(j, 14)], wkt_v[:, j, ts(kk, 128)],
                        R[j][:], start=True, stop=True,
                        skip_group_check=True)
                    nc.tensor.matmul(
                        pab[:, ds(56 + j * 14, 14)],
                        wqt_v[:, j, ts(kk, 128)],
                        R[j][:], start=True, stop=True,
                        skip_group_check=True)
                eng = nc.scalar if kk % 2 else nc.vector
                _copy(eng, AB[kk][:], pab[:])

        # vO (odd-aligned slot pairs): SBUF->SBUF partition-shift DMAs,
        # overlapping the scores sweep (only needed by dwc).
        vO = [pvO.tile([128, TPAD], BF16, name=f"vO{i}", tag=f"vO{i}")
              for i in range(4)]
        for i in range(4):
            nc.scalar.dma_start(vO[i][0:64, :], vT[i][64:128, :])
            nc.scalar.dma_start(vO[i][64:128, :], vT[i + 1][0:64, :])

        # ---- scores sweep + interleaved stage-1 aggregation -----------
        # Per tile tt: scores matmuls + exp/softmax; the agg matmuls and
        # p2 transpose for tile tt-1 are emitted one iteration later so
        # PE never waits on the exp/softmax chain.
        xn_tiles = {}

        def load_xn(tt):
            xn = pxn.tile([128, DIM], BF16, name="xn", tag="xn")
            nc.sync.dma_start(xn[:], xb[ts(tt, 128), :])
            xn_tiles[tt] = xn

        for tt in range(6):
            load_xn(tt)

        with tc.tile_pool(name="sps", bufs=3, space="PSUM") as pps, \
             tc.tile_pool(name="trps", bufs=2, space="PSUM") as ptr, \
             tc.tile_pool(name="a1ps", bufs=1, space="PSUM") as pa1p, \
             tc.tile_pool(name="csps", bufs=1, space="PSUM") as pcsp, \
             tc.tile_pool(name="stmp", bufs=4) as pst:
            a1T = pa1p.tile([128, 8 * 56], F32, name="a1T", tag="a1T")
            pcs = pcsp.tile([56, 1], F32, name="cs", tag="cs")
            p2fs = {}

            def emit_agg(tt):
                for kk in range(8):
                    nc.tensor.matmul(
                        a1T[:, ts(kk, 56)],
                        xn_tiles[tt][:, ts(kk, 128)], u1T[:, ts(tt, 56)],
                        start=(tt == 0), stop=(tt == NT - 1),
                        skip_group_check=True)
                del xn_tiles[tt]
                nc.tensor.matmul(pcs[:], u1T[:, ts(tt, 56)], ones_sb[:],
                                 start=(tt == 0), stop=(tt == NT - 1))

            def emit_p2t(tt):
                ptt = ptr.tile([56, 128], F32, name="p2t", tag="p2t")
                nc.tensor.transpose(ptt[:], p2fs.pop(tt)[:], idf_sb)
                nc.any.tensor_copy(p2T[:, ts(tt, 128)], ptt[:])

            for tt in range(NT + 2):
                if tt < NT:
                    if tt + 6 < NT:
                        load_xn(tt + 6)
                    ps = pps.tile([128, 112], F32, name="s", tag="s")
                    for kk in range(8):
                        nc.tensor.matmul(
                            ps[:], xT(kk)[:, ts(tt, 128)], AB[kk][:],
                            start=(kk == 0), stop=(kk == 7))
                if tt > 0 and tt - 1 < NT:
                    emit_agg(tt - 1)
                if tt > 1:
                    emit_p2t(tt - 2)
                if tt < NT:
                    nc.scalar.activation(
                        u1T[:, ts(tt, 56)], ps[:, 0:56], AF.Exp)
                    u2 = pst.tile([128, 56], F32, name="u2", tag="u2")
                    nc.scalar.activation(u2[:], ps[:, 56:112], AF.Exp)
                    rs = pst.tile([128, 8], F32, name="rs", tag="rs")
                    nc.vector.reduce_sum(
                        rs[:], u2.rearrange("p (h q) -> p h q", q=P),
                        axis=AX.X)
                    nc.vector.reciprocal(rs[:], rs[:])
                    p2f = pst.tile([128, 56], F32, name="p2f", tag="p2f")
                    nc.vector.tensor_tensor(
                        out=p2f.rearrange("p (h q) -> p h q", q=P),
                        in0=u2.rearrange("p (h q) -> p h q", q=P),
                        in1=rs[:, :, None].broadcast_to([128, 8, P]),
                        op=AL.mult,
                    )
                    p2fs[tt] = p2f

            nc.vector.reciprocal(rec1[:], pcs[:])
            nc.scalar.copy(a1sb[:], a1T[:])

        xT_stack.close()  # free xTb SBUF

        # ---- v_agent: va = (a1 @ Wv) * rec, diag blocks -> vabd -------
        # conv_b is folded in here: p2 rows sum to 1 per (token, head), so
        # adding conv_b to the vabd diag blocks makes the fused attention
        # matmul contribute exactly out_att + conv_b.
        with tc.tile_pool(name="vaps", bufs=1, space="PSUM") as pvap, \
             tc.tile_pool(name="vtmp", bufs=1) as pvt:
            pva = pvap.tile([56, 512], F32, name="va", tag="va")
            for kk in range(8):
                nc.tensor.matmul(pva[:, 0:64], a1sb[:, ts(kk, 56)],
                                 wvpa_v[:, kk, ds(64, 64)],
                                 start=(kk == 0), stop=(kk == 7),
                                 skip_group_check=True)
                nc.tensor.matmul(pva[:, 64:512], a1sb[:, ts(kk, 56)],
                                 wvpb_sb[:, kk, ds(0, 448)],
                                 start=(kk == 0), stop=(kk == 7),
                                 skip_group_check=True)
            van = pvt.tile([56, 512], BF16, name="van", tag="van")
            nc.vector.scalar_tensor_tensor(
                out=van[:], in0=pva[:], scalar=rec1[:],
                in1=blob_sb[0:56, OCBT:OCBT + 512],
                op0=AL.mult, op1=AL.add,
            )
            nc.vector.memset(vabd[:], 0.0)
            for h in range(HL):
                nc.sync.dma_start(
                    vabd[ds(P * h, P), ds(64 * h, 64)],
                    van[ds(P * h, P), ds(64 * h, 64)])

        # ---- dwc (depthwise 3x3) + attention output, fused in PSUM ---
        # 8 of 9 taps + the stage-2 attention matmul accumulate in PSUM;
        # the center tap is folded into the PSUM->Y extraction as a fused
        # (src*w + psum) DVE op. conv_b already rides the attention term.
        pY = ctx.enter_context(tc.tile_pool(name="Ypool", bufs=1))
        Y = []
        with tc.tile_pool(name="dwcps", bufs=3, space="PSUM") as pdw:
            for i in range(4):
                src_by_kh = (vT[i], vO[i], vT[i + 1])
                Yi = pY.tile([128, T], BF16, name=f"Y{i}", tag=f"Y{i}")
                Y.append(Yi)
                for tc7 in range(NCH):
                    off = 1 + tc7 * 512
                    pd = pdw.tile([128, 512], F32, name="dwc", tag="dwc")
                    first = True
                    for kh in range(3):
                        for kt in range(3):
                            if kh == 1 and kt == 1:
                                continue    # folded into extraction
                            nc.tensor.matmul(
                                pd[:], taps_v[:, kh * 3 + kt, :],
                                src_by_kh[kh][:, ds(off + kt - 1, 512)],
                                start=first, stop=False)
                            first = False
                    # stage-2 attention output (+conv_b): last accumulation
                    nc.tensor.matmul(
                        pd[:], vabd[:, ts(i, 128)], p2T[:, ts(tc7, 512)],
                        start=False, stop=True)
                    nc.vector.scalar_tensor_tensor(
                        out=Yi[:, ts(tc7, 512)],
                        in0=vO[i][:, ds(off, 512)], scalar=wcol[:, 4:5],
                        in1=pd[:], op0=AL.mult, op1=AL.add)

        # ---- output projection: out = Y^T @ Wo ------------------------
        with tc.tile_pool(name="ostage", bufs=3) as pos, \
             tc.tile_pool(name="ops", bufs=4, space="PSUM") as pop:
            for tt in range(NT):
                po = pop.tile([128, DIM], F32, name="o", tag="o")
                for half in range(2):
                    for k in range(4):
                        nc.tensor.matmul(
                            po[:, ts(half, 512)],
                            Y[k][:, ts(tt, 128)],
                            wop_v[:, k, ts(half, 512)],
                            start=(k == 0), stop=(k == 3))
                ost = pos.tile([128, DIM], BF16, name="ost", tag="ost")
                if tt < NT - 2:
                    _copy(nc.scalar if tt % 2 else nc.vector, ost[:], po[:])
                    nc.gpsimd.dma_start(outp[ts(tt, 128), :], ost[:])
                else:
                    # tail: split copy across engines + low-latency HWDGE
                    nc.vector.tensor_copy(ost[:, 0:512], po[:, 0:512])
                    nc.scalar.copy(ost[:, 512:DIM], po[:, 512:DIM])
                    nc.sync.dma_start(outp[ts(tt, 128), 0:512],
                                      ost[:, 0:512])
                    nc.scalar.dma_start(outp[ts(tt, 128), 512:DIM],
                                        ost[:, 512:DIM])


_NC_CACHE = None


def _get_nc():
    global _NC_CACHE
    if _NC_CACHE is None:
        _NC_CACHE = build_nc()
    return _NC_CACHE


def _prep_core_inputs(x, W_qkv, W_o, conv_w, conv_b):
    bf = ml_dtypes.bfloat16
    ins = []
    # taps[kh*3+kt] = kron(I2, diag(conv_w[:, 0, kh, kt]))
    taps_np = np.zeros((9, 128, 128), dtype=np.float32)
    cw = np.asarray(conv_w, np.float32)
    for kh in range(3):
        for kt in range(3):
            dg = np.diag(cw[:, 0, kh, kt])
            taps_np[kh * 3 + kt, 0:64, 0:64] = dg
            taps_np[kh * 3 + kt, 64:128, 64:128] = dg
    fblob = np.zeros((128, 138), np.float32)
    fblob[:, 0:128] = np.eye(128, dtype=np.float32)
    fblob[:, 128] = np.tile(conv_b, 2)
    for kh in range(3):
        for kt in range(3):
            fblob[:, 129 + kh * 3 + kt] = np.tile(cw[:, 0, kh, kt], 2)

    def pack(w, k):  # [k*128, c] -> [128, k, c]
        c = w.shape[1]
        return np.ascontiguousarray(
            w.reshape(k, 128, c).transpose(1, 0, 2))

    for c in range(8):
        b, g = c // 2, c % 2
        wq = W_qkv[:, 512 * g:512 * g + 512]
        wk = W_qkv[:, 1024 + 512 * g:1024 + 512 * g + 512]
        wv10 = np.zeros((DIM, 640), np.float32)
        for s in range(10):
            h = 8 * g - 1 + s
            if 0 <= h < 16:
                wv10[:, 64 * s:64 * s + 64] = \
                    W_qkv[:, 2048 + 64 * h:2048 + 64 * h + 64]
        blob = np.empty((128, BLOBW), np.float32)
        blob[:, OWQN:OWQN + 4096] = pack(wq, 8).reshape(128, 4096)
        blob[:, OWKT:OWKT + 4096] = pack(
            np.ascontiguousarray(wk.T), 4).reshape(128, 4096)
        blob[:, OWQT:OWQT + 4096] = pack(
            np.ascontiguousarray(wq.T), 4).reshape(128, 4096)
        blob[:, OWOP:OWOP + 4096] = pack(
            np.ascontiguousarray(W_o[512 * g:512 * g + 512, :]),
            4).reshape(128, 4096)
        blob[:, OTAPS:OTAPS + 1152] = taps_np.transpose(1, 0, 2).reshape(
            128, 1152)
        blob[:, OCBT:OCBT + 512] = np.tile(conv_b, 8)[None, :]
        ins.append({
            "xbT": np.ascontiguousarray(x[b].T).astype(bf),
            "xb": np.ascontiguousarray(x[b]).astype(bf),
            "wvpa": pack(wv10[:, 0:128], 8).reshape(128, 1024).astype(bf),
            "wvpb": pack(wv10[:, 128:640], 8).astype(bf),
            "blob": blob.astype(bf),
            "fblob": fblob,
        })
    return ins


def kernel(x, W_qkv, W_o, b_o, conv_w, conv_b, _run_kwargs=None):
    x = np.asarray(x, np.float32)
    W_qkv = np.asarray(W_qkv, np.float32)
    W_o = np.asarray(W_o, np.float32)
    b_o = np.asarray(b_o, np.float32)
    conv_w = np.asarray(conv_w, np.float32)
    conv_b = np.asarray(conv_b, np.float32)

    ins = _prep_core_inputs(x, W_qkv, W_o, conv_w, conv_b)

    nc = _get_nc()
    res = bass_utils.run_bass_kernel_spmd(
        nc, ins, core_ids=list(range(8)), **(_run_kwargs or {}))
    outs = [r["outp"] for r in res.results]
    B = x.shape[0]
    full = np.empty((B, T, DIM), np.float32)
    for b in range(B):
        full[b] = (outs[2 * b].astype(np.float32)
                   + outs[2 * b + 1].astype(np.float32) + b_o[None, :])
    if _run_kwargs:
        kernel.last_results = res
    return full


# revision 31
# speedup vs baseline: 1.8342x; 1.0217x over previous
"""AgentAttention TRN2 kernel: 8 cores = 4 batches x 2 head-groups.

Reference computation (B=4, T=3584, dim=1024, H=16, D=64, P=7):
  qkv = x @ W_qkv -> q,k,v [B,H,T,D]
  agent = avgpool_T(q) [B,H,P,D]
  v_agent = softmax(agent*SC @ k^T) @ v
  out_att = softmax(q*SC @ agent^T) @ v_agent
  dwc = depthwise3x3 over (H,T) of v
  out = (out_att + dwc) 'b h t d -> b t (h d)' @ W_o + b_o

Core c handles batch c//2, heads [8g, 8g+8) with g=c%2. The two partial
outputs per batch are summed on the host (+ b_o).

Structure (all engine-time figures per the TRN2 cost model):
 - q and k are never materialized. Scores contract against x directly:
     s1^T = x @ A^T,  A = (agent*SC) @ Wk^T;   s2 = x @ B,  B = Wq @ agent^T*SC
   agent = (pooled x) @ Wq, with the pooling done as free-dim reduces of
   x^T (host supplies x^T). The only large qkv-side matmul is v
   (10 head slots incl conv halo, 640 cols).
 - Stage-1 aggregation re-associated through x: va = (u1^T @ x) @ Wv,
   emitted transposed (a1^T accumulated in one PSUM bank, 8 col-groups)
   and interleaved tile-by-tile into the scores sweep.
 - The stage-2 attention output matmul accumulates into the depthwise
   conv PSUM tile as a 10th accumulation step.
 - DMA transfers and HWDGE generation are each globally serialized in
   the cost model, so: weights are packed host-side into one blob DMA,
   x^T arrives as 32 column-slab DMAs ordered so PE can start after the
   first slab, x-natural streams through a rolling pool during the
   scores sweep, and the output is written bf16, one DMA per token tile
   on the software-DGE (Pool) path to keep HWDGE free.
"""

import numpy as np
import ml_dtypes

import concourse.bass as bass
import concourse.bacc as bacc
import concourse.mybir as mybir
import concourse.tile as tile
from concourse.bass import ts, ds
from concourse import bass_utils

F32 = mybir.dt.float32
BF16 = mybir.dt.bfloat16
AX = mybir.AxisListType
AF = mybir.ActivationFunctionType
AL = mybir.AluOpType

T, DIM, D, P = 3584, 1024, 64, 7
HL = 8                      # local heads per core
SC = D ** -0.5
NT = T // 128               # 28 token tiles of 128
NCH = T // 512              # 7 chunks of 512
TPAD = T + 2                # vT padded with one zero col each side

# blob layout (bf16, per-partition column offsets)
OWQN, OWKT, OWQT, OWOP, OTAPS = 0, 4096, 8192, 12288, 16384
OCBT = OTAPS + 9 * 128      # conv_b tiled row, replicated on all partitions
BLOBW = OCBT + 512          # 18048


def build_nc(skip=()):
    # Bacc (not plain Bass): its compile() runs generate_event_semaphores,
    # which splits multi-wait sync_info into InstEventSemaphore -- TRN2
    # instructions can carry at most one embedded wait.
    nc = bacc.Bacc("TRN2", target_bir_lowering=False)

    xbT = nc.dram_tensor("xbT", [DIM, T], BF16, kind="ExternalInput")
    xb = nc.dram_tensor("xb", [T, DIM], BF16, kind="ExternalInput")
    wvpa = nc.dram_tensor("wvpa", [128, 1024], BF16, kind="ExternalInput")
    wvpb = nc.dram_tensor("wvpb", [128, 8, 512], BF16, kind="ExternalInput")
    blob = nc.dram_tensor("blob", [128, BLOBW], BF16, kind="ExternalInput")
    fblob = nc.dram_tensor("fblob", [128, 138], F32, kind="ExternalInput")
    outp = nc.dram_tensor("outp", [T, DIM], BF16, kind="ExternalOutput")

    with tile.TileContext(nc) as tc:
        _emit(nc, tc, xbT, xb, wvpa, wvpb, blob, fblob, outp)
    nc.compile()
    return nc


def _copy(eng, out, in_):
    if hasattr(eng, "activation"):
        eng.copy(out, in_)
    else:
        eng.tensor_copy(out, in_)


def _emit(nc, tc, xbT, xb, wvpa, wvpb, blob, fblob, outp):
    import contextlib
    ctx = contextlib.ExitStack()
    with ctx:
        # ---- constants + weight blob ----------------------------------
        pconst = ctx.enter_context(tc.tile_pool(name="const", bufs=1))
        fb_sb = pconst.tile([128, 138], F32, name="fb", tag="fb")
        nc.sync.dma_start(fb_sb[:], fblob[:])
        idf_sb = fb_sb[:, 0:128]
        wcol = fb_sb[:, 129:138]    # per-partition dwc tap weights
        ones_sb = pconst.tile([128, 1], BF16, name="ones", tag="ones")
        nc.vector.memset(ones_sb[:], 1.0)

        pwvp = ctx.enter_context(tc.tile_pool(name="wvp", bufs=1))
        wvpa_sb = pwvp.tile([128, 1024], BF16, name="wvpa", tag="wvpa")
        nc.sync.dma_start(wvpa_sb[:], wvpa[:])
        wvpa_v = wvpa_sb.rearrange("p (k c) -> p k c", c=128)
        wvpb_sb = pwvp.tile([128, 8, 512], BF16, name="wvpb", tag="wvpb")

        def wv_lhs(kk, cg):
            if cg == 0:
                return wvpa_v[:, kk, :]
            return wvpb_sb[:, kk, ds((cg - 1) * 128, 128)]

        psmall = ctx.enter_context(tc.tile_pool(name="small", bufs=1))
        xsumT = [psmall.tile([128, P], F32, name=f"xsT{k}", tag=f"xsT{k}")
                 for k in range(8)]
        xsumB = [psmall.tile([128, P], BF16, name=f"xsB{k}", tag=f"xsB{k}")
                 for k in range(8)]
        R = [psmall.tile([128, 2 * P], BF16, name=f"R{j}", tag=f"R{j}")
             for j in range(4)]
        AB = [psmall.tile([128, 112], BF16, name=f"AB{k}", tag=f"AB{k}")
              for k in range(8)]
        u1T = psmall.tile([128, NT * 56], BF16, name="u1T", tag="u1T")
        p2T = psmall.tile([56, T], BF16, name="p2T", tag="p2T")
        a1sb = psmall.tile([128, 8 * 56], BF16, name="a1sb", tag="a1sb")
        vabd = psmall.tile([56, 512], BF16, name="vabd", tag="vabd")
        rec1 = psmall.tile([56, 1], F32, name="rec1", tag="rec1")

        # ---- vT: v (10 head slots incl halo) transposed, bf16, t-padded
        pvT = ctx.enter_context(tc.tile_pool(name="vT", bufs=1))
        vT = [pvT.tile([128, TPAD], BF16, name=f"vT{j}", tag=f"vT{j}")
              for j in range(5)]
        for j in range(5):
            nc.vector.memset(vT[j][:, 0:1], 0.0)
            nc.vector.memset(vT[j][:, TPAD - 1:TPAD], 0.0)

        # pools that outlive xT must be created before it (LIFO release)
        pblob = ctx.enter_context(tc.tile_pool(name="blob", bufs=1))
        pvO = ctx.enter_context(tc.tile_pool(name="vO", bufs=1))
        pxn = ctx.enter_context(tc.tile_pool(name="xnat", bufs=8))

        # ---- phase 1: xT slab loads + v matmul ------------------------
        # xT arrives in 4 column slabs x 8 kk tiles; v-matmul chains are
        # ordered by slab so PE starts once slab 0 lands.
        import contextlib as _cl
        xT_stack = _cl.ExitStack()
        pxT = xT_stack.enter_context(tc.tile_pool(name="xT", bufs=1))
        xTb = pxT.tile([128, 8 * T], BF16, name="xTb", tag="xTb")

        def xT(kk):
            return xTb[:, ds(kk * T, T)]

        SLABS = [(0, 512, [0]), (512, 512, [1]), (1024, 1024, [2, 3]),
                 (2048, 1024, [4, 5]), (3072, 512, [6])]
        slab_chunks = []
        for si, (off, w, chunks) in enumerate(SLABS):
            for kk in range(8):
                nc.sync.dma_start(
                    xTb[:, ds(kk * T + off, w)],
                    xbT[ts(kk, 128), ds(off, w)])
            if si == 0:
                nc.sync.dma_start(wvpb_sb[:], wvpb[:])
            slab_chunks.append(chunks)

        with tc.tile_pool(name="mmps", bufs=4, space="PSUM") as pmm:
            nmm = 0
            for chunks in slab_chunks:
                for cg in range(5):
                    for ch in chunks:
                        pm = pmm.tile([128, 512], F32, name="mm", tag="mm")
                        for kk in range(8):
                            nc.tensor.matmul(
                                pm[:], wv_lhs(kk, cg),
                                xT(kk)[:, ds(ch * 512, 512)],
                                start=(kk == 0), stop=(kk == 7),
                            )
                        eng = nc.scalar if nmm % 2 else nc.vector
                        nmm += 1
                        _copy(eng, vT[cg][:, ds(1 + ch * 512, 512)], pm[:])
                # xsum reduces for this slab's chunks (keeps DVE current so
                # the agent/AB chain isn't stuck behind late vT copies)
                for ch in chunks:
                    for kk in range(8):
                        nc.vector.reduce_sum(
                            xsumT[kk][:, ch:ch + 1],
                            xT(kk)[:, ds(ch * 512, 512)], axis=AX.X)

        # weight blob (queued on sync after the xT slabs)
        blob_sb = pblob.tile([128, BLOBW], BF16, name="blob", tag="blob")
        nc.sync.dma_start(blob_sb[:], blob[:])
        wqn_v = blob_sb[:, OWQN:OWQN + 4096].rearrange(
            "p (k c) -> p k c", c=512)
        wkt_v = blob_sb[:, OWKT:OWKT + 4096].rearrange(
            "p (k c) -> p k c", c=1024)
        wqt_v = blob_sb[:, OWQT:OWQT + 4096].rearrange(
            "p (k c) -> p k c", c=1024)
        wop_v = blob_sb[:, OWOP:OWOP + 4096].rearrange(
            "p (k c) -> p k c", c=1024)
        taps_v = blob_sb[:, OTAPS:OTAPS + 1152].rearrange(
            "p (k c) -> p k c", c=128)

        # xsum -> bf16 for the agent matmuls
        for kk in range(8):
            nc.scalar.copy(xsumB[kk][:], xsumT[kk][:])

        # ---- agent + A/B formation ------------------------------------
        with tc.tile_pool(name="agps", bufs=1, space="PSUM") as pagp, \
             tc.tile_pool(name="abps", bufs=2, space="PSUM") as pabp:
            agJ = [pagp.tile([128, P], F32, name=f"agJ{j}", tag=f"agJ{j}")
                   for j in range(4)]
            for j in range(4):
                for kk in range(8):
                    nc.tensor.matmul(
                        agJ[j][:], wqn_v[:, kk, ds(j * 128, 128)],
                        xsumB[kk][:], start=(kk == 0), stop=(kk == 7))
            # R[j]: block-diagonal agent^T * SC/512, bf16 [128, 14]
            for j in range(4):
                nc.vector.memset(R[j][:], 0.0)
                nc.scalar.activation(
                    R[j][0:64, 0:P], agJ[j][0:64, :], AF.Copy,
                    scale=SC / 512.0)
                nc.scalar.activation(
                    R[j][64:128, P:2 * P], agJ[j][64:128, :], AF.Copy,
                    scale=SC / 512.0)
            # AB[kk] = [A^T | B] slice [128 m, 112]
            for kk in range(8):
                pab = pabp.tile([128, 112], F32, name="ab", tag="ab")
                for j in range(4):
                    nc.tensor.matmul(
                        pab[:, ts(j, 14)], wkt_v[:, j, ts(kk, 128)],
                        R[j][:], start=True, stop=True,
                        skip_group_check=True)
                    nc.tensor.matmul(
                        pab[:, ds(56 + j * 14, 14)],
                        wqt_v[:, j, ts(kk, 128)],
                        R[j][:], start=True, stop=True,
                        skip_group_check=True)
                eng = nc.scalar if kk % 2 else nc.vector
                _copy(eng, AB[kk][:], pab[:])

        # vO (odd-aligned slot pairs): SBUF->SBUF partition-shift DMAs,
        # overlapping the scores sweep (only needed by dwc).
        vO = [pvO.tile([128, TPAD], BF16, name=f"vO{i}", tag=f"vO{i}")
              for i in range(4)]
        for i in range(4):
            nc.scalar.dma_start(vO[i][0:64, :], vT[i][64:128, :])
            nc.scalar.dma_start(vO[i][64:128, :], vT[i + 1][0:64, :])

        # ---- scores sweep + interleaved stage-1 aggregation -----------
        # Per tile tt: scores matmuls + exp/softmax; the agg matmuls and
        # p2 transpose for tile tt-1 are emitted one iteration later so
        # PE never waits on the exp/softmax chain.
        xn_tiles = {}

        def load_xn(tt):
            xn = pxn.tile([128, DIM], BF16, name="xn", tag="xn")
            nc.sync.dma_start(xn[:], xb[ts(tt, 128), :])
            xn_tiles[tt] = xn

        for tt in range(6):
            load_xn(tt)

        with tc.tile_pool(name="sps", bufs=3, space="PSUM") as pps, \
             tc.tile_pool(name="trps", bufs=2, space="PSUM") as ptr, \
             tc.tile_pool(name="a1ps", bufs=1, space="PSUM") as pa1p, \
             tc.tile_pool(name="csps", bufs=1, space="PSUM") as pcsp, \
             tc.tile_pool(name="stmp", bufs=4) as pst:
            a1T = pa1p.tile([128, 8 * 56], F32, name="a1T", tag="a1T")
            pcs = pcsp.tile([56, 1], F32, name="cs", tag="cs")
            p2fs = {}

            def emit_agg(tt):
                for kk in range(8):
                    nc.tensor.matmul(
                        a1T[:, ts(kk, 56)],
                        xn_tiles[tt][:, ts(kk, 128)], u1T[:, ts(tt, 56)],
                        start=(tt == 0), stop=(tt == NT - 1),
                        skip_group_check=True)
                del xn_tiles[tt]
                nc.tensor.matmul(pcs[:], u1T[:, ts(tt, 56)], ones_sb[:],
                                 start=(tt == 0), stop=(tt == NT - 1))

            def emit_p2t(tt):
                ptt = ptr.tile([56, 128], F32, name="p2t", tag="p2t")
                nc.tensor.transpose(ptt[:], p2fs.pop(tt)[:], idf_sb)
                nc.any.tensor_copy(p2T[:, ts(tt, 128)], ptt[:])

            for tt in range(NT + 2):
                if tt < NT:
                    if tt + 6 < NT:
                        load_xn(tt + 6)
                    ps = pps.tile([128, 112], F32, name="s", tag="s")
                    for kk in range(8):
                        nc.tensor.matmul(
                            ps[:], xT(kk)[:, ts(tt, 128)], AB[kk][:],
                            start=(kk == 0), stop=(kk == 7))
                if tt > 0 and tt - 1 < NT:
                    emit_agg(tt - 1)
                if tt > 1:
                    emit_p2t(tt - 2)
                if tt < NT:
                    nc.scalar.activation(
                        u1T[:, ts(tt, 56)], ps[:, 0:56], AF.Exp)
                    u2 = pst.tile([128, 56], F32, name="u2", tag="u2")
                    nc.scalar.activation(u2[:], ps[:, 56:112], AF.Exp)
                    rs = pst.tile([128, 8], F32, name="rs", tag="rs")
                    nc.vector.reduce_sum(
                        rs[:], u2.rearrange("p (h q) -> p h q", q=P),
                        axis=AX.X)
                    nc.vector.reciprocal(rs[:], rs[:])
                    p2f = pst.tile([128, 56], F32, name="p2f", tag="p2f")
                    nc.vector.tensor_tensor(
                        out=p2f.rearrange("p (h q) -> p h q", q=P),
                        in0=u2.rearrange("p (h q) -> p h q", q=P),
                        in1=rs[:, :, None].broadcast_to([128, 8, P]),
                        op=AL.mult,
                    )
                    p2fs[tt] = p2f

            nc.vector.reciprocal(rec1[:], pcs[:])
            nc.scalar.copy(a1sb[:], a1T[:])

        xT_stack.close()  # free xTb SBUF

        # ---- v_agent: va = (a1 @ Wv) * rec, diag blocks -> vabd -------
        # conv_b is folded in here: p2 rows sum to 1 per (token, head), so
        # adding conv_b to the vabd diag blocks makes the fused attention
        # matmul contribute exactly out_att + conv_b.
        with tc.tile_pool(name="vaps", bufs=1, space="PSUM") as pvap, \
             tc.tile_pool(name="vtmp", bufs=1) as pvt:
            pva = pvap.tile([56, 512], F32, name="va", tag="va")
            for kk in range(8):
                nc.tensor.matmul(pva[:, 0:64], a1sb[:, ts(kk, 56)],
                                 wvpa_v[:, kk, ds(64, 64)],
                                 start=(kk == 0), stop=(kk == 7),
                                 skip_group_check=True)
                nc.tensor.matmul(pva[:, 64:512], a1sb[:, ts(kk, 56)],
                                 wvpb_sb[:, kk, ds(0, 448)],
                                 start=(kk == 0), stop=(kk == 7),
                                 skip_group_check=True)
            van = pvt.tile([56, 512], BF16, name="van", tag="van")
            nc.vector.scalar_tensor_tensor(
                out=van[:], in0=pva[:], scalar=rec1[:],
                in1=blob_sb[0:56, OCBT:OCBT + 512],
                op0=AL.mult, op1=AL.add,
            )
            nc.vector.memset(vabd[:], 0.0)
            for h in range(HL):
                nc.sync.dma_start(
                    vabd[ds(P * h, P), ds(64 * h, 64)],
                    van[ds(P * h, P), ds(64 * h, 64)])

        # ---- dwc (depthwise 3x3) + attention output, fused in PSUM ---
        # 8 of 9 taps + the stage-2 attention matmul accumulate in PSUM;
        # the center tap is folded into the PSUM->Y extraction as a fused
        # (src*w + psum) DVE op. conv_b already rides the attention term.
        pY = ctx.enter_context(tc.tile_pool(name="Ypool", bufs=1))
        Y = []
        with tc.tile_pool(name="dwcps", bufs=3, space="PSUM") as pdw:
            for i in range(4):
                src_by_kh = (vT[i], vO[i], vT[i + 1])
                Yi = pY.tile([128, T], BF16, name=f"Y{i}", tag=f"Y{i}")
                Y.append(Yi)
                for tc7 in range(NCH):
                    off = 1 + tc7 * 512
                    pd = pdw.tile([128, 512], F32, name="dwc", tag="dwc")
                    first = True
                    for kh, kt in ((0, 0), (0, 1), (0, 2), (1, 2),
                                   (2, 0), (2, 1), (2, 2)):
                        nc.tensor.matmul(
                            pd[:], taps_v[:, kh * 3 + kt, :],
                            src_by_kh[kh][:, ds(off + kt - 1, 512)],
                            start=first, stop=False)
                        first = False
                    # stage-2 attention output (+conv_b): last accumulation
                    nc.tensor.matmul(
                        pd[:], vabd[:, ts(i, 128)], p2T[:, ts(tc7, 512)],
                        start=False, stop=True)
                    yslc = Yi[:, ts(tc7, 512)]
                    nc.vector.scalar_tensor_tensor(
                        out=yslc, in0=vO[i][:, ds(off, 512)],
                        scalar=wcol[:, 4:5], in1=pd[:],
                        op0=AL.mult, op1=AL.add)
                    nc.vector.scalar_tensor_tensor(
                        out=yslc, in0=vO[i][:, ds(off - 1, 512)],
                        scalar=wcol[:, 3:4], in1=yslc,
                        op0=AL.mult, op1=AL.add)

        # ---- output projection: out = Y^T @ Wo ------------------------
        with tc.tile_pool(name="ostage", bufs=3) as pos, \
             tc.tile_pool(name="ops", bufs=4, space="PSUM") as pop:
            for tt in range(NT):
                po = pop.tile([128, DIM], F32, name="o", tag="o")
                for half in range(2):
                    for k in range(4):
                        nc.tensor.matmul(
                            po[:, ts(half, 512)],
                            Y[k][:, ts(tt, 128)],
                            wop_v[:, k, ts(half, 512)],
                            start=(k == 0), stop=(k == 3))
                ost = pos.tile([128, DIM], BF16, name="ost", tag="ost")
                if tt < NT - 2:
                    _copy(nc.scalar if tt % 2 else nc.vector, ost[:], po[:])
                    nc.gpsimd.dma_start(outp[ts(tt, 128), :], ost[:])
                elif tt == NT - 2:
                    nc.scalar.copy(ost[:], po[:])
                    nc.scalar.dma_start(outp[ts(tt, 128), :], ost[:])
                else:   # last tile: low-latency HWDGE path
                    nc.vector.tensor_copy(ost[:], po[:])
                    nc.sync.dma_start(outp[ts(tt, 128), :], ost[:])


_NC_CACHE = None


def _get_nc():
    global _NC_CACHE
    if _NC_CACHE is None:
        _NC_CACHE = build_nc()
    return _NC_CACHE


def _prep_core_inputs(x, W_qkv, W_o, conv_w, conv_b):
    bf = ml_dtypes.bfloat16
    ins = []
    # taps[kh*3+kt] = kron(I2, diag(conv_w[:, 0, kh, kt]))
    taps_np = np.zeros((9, 128, 128), dtype=np.float32)
    cw = np.asarray(conv_w, np.float32)
    for kh in range(3):
        for kt in range(3):
            dg = np.diag(cw[:, 0, kh, kt])
            taps_np[kh * 3 + kt, 0:64, 0:64] = dg
            taps_np[kh * 3 + kt, 64:128, 64:128] = dg
    fblob = np.zeros((128, 138), np.float32)
    fblob[:, 0:128] = np.eye(128, dtype=np.float32)
    fblob[:, 128] = np.tile(conv_b, 2)
    for kh in range(3):
        for kt in range(3):
            fblob[:, 129 + kh * 3 + kt] = np.tile(cw[:, 0, kh, kt], 2)

    def pack(w, k):  # [k*128, c] -> [128, k, c]
        c = w.shape[1]
        return np.ascontiguousarray(
            w.reshape(k, 128, c).transpose(1, 0, 2))

    for c in range(8):
        b, g = c // 2, c % 2
        wq = W_qkv[:, 512 * g:512 * g + 512]
        wk = W_qkv[:, 1024 + 512 * g:1024 + 512 * g + 512]
        wv10 = np.zeros((DIM, 640), np.float32)
        for s in range(10):
            h = 8 * g - 1 + s
            if 0 <= h < 16:
                wv10[:, 64 * s:64 * s + 64] = \
                    W_qkv[:, 2048 + 64 * h:2048 + 64 * h + 64]
        blob = np.empty((128, BLOBW), np.float32)
        blob[:, OWQN:OWQN + 4096] = pack(wq, 8).reshape(128, 4096)
        blob[:, OWKT:OWKT + 4096] = pack(
            np.ascontiguousarray(wk.T), 4).reshape(128, 4096)
        blob[:, OWQT:OWQT + 4096] = pack(
            np.ascontiguousarray(wq.T), 4).reshape(128, 4096)
        blob[:, OWOP:OWOP + 4096] = pack(
            np.ascontiguousarray(W_o[512 * g:512 * g + 512, :]),
            4).reshape(128, 4096)
        blob[:, OTAPS:OTAPS + 1152] = taps_np.transpose(1, 0, 2).reshape(
            128, 1152)
        blob[:, OCBT:OCBT + 512] = np.tile(conv_b, 8)[None, :]
        ins.append({
            "xbT": np.ascontiguousarray(x[b].T).astype(bf),
            "xb": np.ascontiguousarray(x[b]).astype(bf),
            "wvpa": pack(wv10[:, 0:128], 8).reshape(128, 1024).astype(bf),
            "wvpb": pack(wv10[:, 128:640], 8).astype(bf),
            "blob": blob.astype(bf),
            "fblob": fblob,
        })
    return ins


def kernel(x, W_qkv, W_o, b_o, conv_w, conv_b, _run_kwargs=None):
    x = np.asarray(x, np.float32)
    W_qkv = np.asarray(W_qkv, np.float32)
    W_o = np.asarray(W_o, np.float32)
    b_o = np.asarray(b_o, np.float32)
    conv_w = np.asarray(conv_w, np.float32)
    conv_b = np.asarray(conv_b, np.float32)

    ins = _prep_core_inputs(x, W_qkv, W_o, conv_w, conv_b)

    nc = _get_nc()
    res = bass_utils.run_bass_kernel_spmd(
        nc, ins, core_ids=list(range(8)), **(_run_kwargs or {}))
    outs = [r["outp"] for r in res.results]
    B = x.shape[0]
    full = np.empty((B, T, DIM), np.float32)
    for b in range(B):
        full[b] = (outs[2 * b].astype(np.float32)
                   + outs[2 * b + 1].astype(np.float32) + b_o[None, :])
    if _run_kwargs:
        kernel.last_results = res
    return full


# revision 40
# speedup vs baseline: 1.8441x; 1.0054x over previous
"""AgentAttention TRN2 kernel: 8 cores = 4 batches x 2 head-groups.

Reference computation (B=4, T=3584, dim=1024, H=16, D=64, P=7):
  qkv = x @ W_qkv -> q,k,v [B,H,T,D]
  agent = avgpool_T(q) [B,H,P,D]
  v_agent = softmax(agent*SC @ k^T) @ v
  out_att = softmax(q*SC @ agent^T) @ v_agent
  dwc = depthwise3x3 over (H,T) of v
  out = (out_att + dwc) 'b h t d -> b t (h d)' @ W_o + b_o

Core c handles batch c//2, heads [8g, 8g+8) with g=c%2. The two partial
outputs per batch are summed on the host (+ b_o).

Structure (all engine-time figures per the TRN2 cost model):
 - q and k are never materialized. Scores contract against x directly:
     s1^T = x @ A^T,  A = (agent*SC) @ Wk^T;   s2 = x @ B,  B = Wq @ agent^T*SC
   agent = (pooled x) @ Wq, with the pooling done as free-dim reduces of
   x^T (host supplies x^T). The only large qkv-side matmul is v
   (10 head slots incl conv halo, 640 cols).
 - Stage-1 aggregation re-associated through x: va = (u1^T @ x) @ Wv,
   emitted transposed (a1^T accumulated in one PSUM bank, 8 col-groups)
   and interleaved tile-by-tile into the scores sweep.
 - The stage-2 attention output matmul accumulates into the depthwise
   conv PSUM tile as a 10th accumulation step.
 - DMA transfers and HWDGE generation are each globally serialized in
   the cost model, so: weights are packed host-side into one blob DMA,
   x^T arrives as 32 column-slab DMAs ordered so PE can start after the
   first slab, x-natural streams through a rolling pool during the
   scores sweep, and the output is written bf16, one DMA per token tile
   on the software-DGE (Pool) path to keep HWDGE free.
"""

import numpy as np
import ml_dtypes

import concourse.bass as bass
import concourse.bacc as bacc
import concourse.mybir as mybir
import concourse.tile as tile
from concourse.bass import ts, ds
from concourse import bass_utils

F32 = mybir.dt.float32
BF16 = mybir.dt.bfloat16
AX = mybir.AxisListType
AF = mybir.ActivationFunctionType
AL = mybir.AluOpType

T, DIM, D, P = 3584, 1024, 64, 7
HL = 8                      # local heads per core
SC = D ** -0.5
NT = T // 128               # 28 token tiles of 128
NCH = T // 512              # 7 chunks of 512
TPAD = T + 2                # vT padded with one zero col each side

# blob layout (bf16, per-partition column offsets)
OWQN, OWKT, OWQT, OWOP, OTAPS = 0, 4096, 8192, 12288, 16384
OCBT = OTAPS + 9 * 128      # conv_b tiled row, replicated on all partitions
OIDB = OCBT + 512           # bf16 identity (for p2 transposes)
BLOBW = OIDB + 128          # 18176


def build_nc(skip=()):
    # Bacc (not plain Bass): its compile() runs generate_event_semaphores,
    # which splits multi-wait sync_info into InstEventSemaphore -- TRN2
    # instructions can carry at most one embedded wait.
    nc = bacc.Bacc("TRN2", target_bir_lowering=False)

    xbT = nc.dram_tensor("xbT", [DIM, T], BF16, kind="ExternalInput")
    xb = nc.dram_tensor("xb", [T, DIM], BF16, kind="ExternalInput")
    wvpa = nc.dram_tensor("wvpa", [128, 1024], BF16, kind="ExternalInput")
    wvpb = nc.dram_tensor("wvpb", [128, 8, 512], BF16, kind="ExternalInput")
    blob = nc.dram_tensor("blob", [128, BLOBW], BF16, kind="ExternalInput")
    fblob = nc.dram_tensor("fblob", [128, 138], F32, kind="ExternalInput")
    outp = nc.dram_tensor("outp", [T, DIM], BF16, kind="ExternalOutput")

    with tile.TileContext(nc) as tc:
        _emit(nc, tc, xbT, xb, wvpa, wvpb, blob, fblob, outp)
    nc.compile()
    return nc


def _copy(eng, out, in_):
    if hasattr(eng, "activation"):
        eng.copy(out, in_)
    else:
        eng.tensor_copy(out, in_)


def _emit(nc, tc, xbT, xb, wvpa, wvpb, blob, fblob, outp):
    import contextlib
    ctx = contextlib.ExitStack()
    with ctx:
        # ---- constants + weight blob ----------------------------------
        pconst = ctx.enter_context(tc.tile_pool(name="const", bufs=1))
        fb_sb = pconst.tile([128, 138], F32, name="fb", tag="fb")
        nc.sync.dma_start(fb_sb[:], fblob[:])
        idf_sb = fb_sb[:, 0:128]
        wcol = fb_sb[:, 129:138]    # per-partition dwc tap weights
        ones_sb = pconst.tile([128, 1], BF16, name="ones", tag="ones")
        nc.vector.memset(ones_sb[:], 1.0)

        pwvp = ctx.enter_context(tc.tile_pool(name="wvp", bufs=1))
        wvpa_sb = pwvp.tile([128, 1024], BF16, name="wvpa", tag="wvpa")
        nc.sync.dma_start(wvpa_sb[:], wvpa[:])
        wvpa_v = wvpa_sb.rearrange("p (k c) -> p k c", c=128)
        wvpb_sb = pwvp.tile([128, 8, 512], BF16, name="wvpb", tag="wvpb")
        nc.sync.dma_start(wvpb_sb[:], wvpb[:])

        def wv_lhs(kk, cg):
            if cg == 0:
                return wvpa_v[:, kk, :]
            return wvpb_sb[:, kk, ds((cg - 1) * 128, 128)]

        psmall = ctx.enter_context(tc.tile_pool(name="small", bufs=1))
        xsumT = [psmall.tile([128, P], F32, name=f"xsT{k}", tag=f"xsT{k}")
                 for k in range(8)]
        xsumB = [psmall.tile([128, P], BF16, name=f"xsB{k}", tag=f"xsB{k}")
                 for k in range(8)]
        R = [psmall.tile([128, 2 * P], BF16, name=f"R{j}", tag=f"R{j}")
             for j in range(4)]
        AB = [psmall.tile([128, 112], BF16, name=f"AB{k}", tag=f"AB{k}")
              for k in range(8)]
        u1T = psmall.tile([128, NT * 56], BF16, name="u1T", tag="u1T")
        p2T = psmall.tile([56, T], BF16, name="p2T", tag="p2T")
        a1sb = psmall.tile([128, 8 * 56], BF16, name="a1sb", tag="a1sb")
        vabd = psmall.tile([56, 512], BF16, name="vabd", tag="vabd")
        rec1 = psmall.tile([56, 1], F32, name="rec1", tag="rec1")

        # ---- vT: v (10 head slots incl halo) transposed, bf16, t-padded
        pvT = ctx.enter_context(tc.tile_pool(name="vT", bufs=1))
        vT = [pvT.tile([128, TPAD], BF16, name=f"vT{j}", tag=f"vT{j}")
              for j in range(5)]
        for j in range(5):
            nc.vector.memset(vT[j][:, 0:1], 0.0)
            nc.vector.memset(vT[j][:, TPAD - 1:TPAD], 0.0)

        # pools that outlive xT must be created before it (LIFO release)
        pblob = ctx.enter_context(tc.tile_pool(name="blob", bufs=1))
        pvO = ctx.enter_context(tc.tile_pool(name="vO", bufs=1))
        pxn = ctx.enter_context(tc.tile_pool(name="xnat", bufs=8))

        # ---- phase 1: xT slab loads + v matmul ------------------------
        # xT arrives in 4 column slabs x 8 kk tiles; v-matmul chains are
        # ordered by slab so PE starts once slab 0 lands.
        import contextlib as _cl
        xT_stack = _cl.ExitStack()
        pxT = xT_stack.enter_context(tc.tile_pool(name="xT", bufs=1))
        xTb = pxT.tile([128, 8 * T], BF16, name="xTb", tag="xTb")

        def xT(kk):
            return xTb[:, ds(kk * T, T)]

        SLABS = [(0, 512, [0]), (512, 512, [1]), (1024, 1024, [2, 3]),
                 (2048, 1024, [4, 5]), (3072, 512, [6])]
        slab_chunks = []
        for si, (off, w, chunks) in enumerate(SLABS):
            for kk in range(8):
                nc.sync.dma_start(
                    xTb[:, ds(kk * T + off, w)],
                    xbT[ts(kk, 128), ds(off, w)])
            slab_chunks.append(chunks)

        with tc.tile_pool(name="mmps", bufs=4, space="PSUM") as pmm:
            nmm = 0
            for chunks in slab_chunks:
                for cg in range(5):
                    for ch in chunks:
                        pm = pmm.tile([128, 512], F32, name="mm", tag="mm")
                        for kk in range(8):
                            nc.tensor.matmul(
                                pm[:], wv_lhs(kk, cg),
                                xT(kk)[:, ds(ch * 512, 512)],
                                start=(kk == 0), stop=(kk == 7),
                            )
                        eng = nc.scalar if nmm % 2 else nc.vector
                        nmm += 1
                        _copy(eng, vT[cg][:, ds(1 + ch * 512, 512)], pm[:])
                # xsum reduces for this slab's chunks (keeps DVE current so
                # the agent/AB chain isn't stuck behind late vT copies)
                for ch in chunks:
                    for kk in range(8):
                        nc.vector.reduce_sum(
                            xsumT[kk][:, ch:ch + 1],
                            xT(kk)[:, ds(ch * 512, 512)], axis=AX.X)

        # weight blob (queued on sync after the xT slabs)
        blob_sb = pblob.tile([128, BLOBW], BF16, name="blob", tag="blob")
        nc.sync.dma_start(blob_sb[:], blob[:])
        wqn_v = blob_sb[:, OWQN:OWQN + 4096].rearrange(
            "p (k c) -> p k c", c=512)
        wkt_v = blob_sb[:, OWKT:OWKT + 4096].rearrange(
            "p (k c) -> p k c", c=1024)
        wqt_v = blob_sb[:, OWQT:OWQT + 4096].rearrange(
            "p (k c) -> p k c", c=1024)
        wop_v = blob_sb[:, OWOP:OWOP + 4096].rearrange(
            "p (k c) -> p k c", c=1024)
        taps_v = blob_sb[:, OTAPS:OTAPS + 1152].rearrange(
            "p (k c) -> p k c", c=128)
        idb_v = blob_sb[:, OIDB:OIDB + 128]

        # xsum -> bf16 for the agent matmuls
        for kk in range(8):
            nc.scalar.copy(xsumB[kk][:], xsumT[kk][:])

        # ---- agent + A/B formation ------------------------------------
        with tc.tile_pool(name="agps", bufs=1, space="PSUM") as pagp, \
             tc.tile_pool(name="abps", bufs=4, space="PSUM") as pabp:
            agJ = [pagp.tile([128, P], F32, name=f"agJ{j}", tag=f"agJ{j}")
                   for j in range(4)]
            for j in range(4):
                for kk in range(8):
                    nc.tensor.matmul(
                        agJ[j][:], wqn_v[:, kk, ds(j * 128, 128)],
                        xsumB[kk][:], start=(kk == 0), stop=(kk == 7))
            # R[j]: block-diagonal agent^T * SC/512, bf16 [128, 14]
            for j in range(4):
                nc.vector.memset(R[j][:], 0.0)
                nc.scalar.activation(
                    R[j][0:64, 0:P], agJ[j][0:64, :], AF.Copy,
                    scale=SC / 512.0)
                nc.scalar.activation(
                    R[j][64:128, P:2 * P], agJ[j][64:128, :], AF.Copy,
                    scale=SC / 512.0)
            # AB[kk] = [A^T | B] slice [128 m, 112]
            for kk in range(8):
                pab = pabp.tile([128, 112], F32, name="ab", tag="ab")
                for j in range(4):
                    nc.tensor.matmul(
                        pab[:, ts(j, 14)], wkt_v[:, j, ts(kk, 128)],
                        R[j][:], start=True, stop=True,
                        skip_group_check=True)
                    nc.tensor.matmul(
                        pab[:, ds(56 + j * 14, 14)],
                        wqt_v[:, j, ts(kk, 128)],
                        R[j][:], start=True, stop=True,
                        skip_group_check=True)
                eng = nc.scalar if kk % 2 else nc.vector
                _copy(eng, AB[kk][:], pab[:])

        # vO (odd-aligned slot pairs): SBUF->SBUF partition-shift DMAs,
        # overlapping the scores sweep (only needed by dwc).
        vO = [pvO.tile([128, TPAD], BF16, name=f"vO{i}", tag=f"vO{i}")
              for i in range(4)]
        for i in range(4):
            nc.scalar.dma_start(vO[i][0:64, :], vT[i][64:128, :])
            nc.scalar.dma_start(vO[i][64:128, :], vT[i + 1][0:64, :])

        # ---- scores sweep + interleaved stage-1 aggregation -----------
        # Per tile tt: scores matmuls + exp/softmax; the agg matmuls and
        # p2 transpose for tile tt-1 are emitted one iteration later so
        # PE never waits on the exp/softmax chain.
        xn_tiles = {}

        def load_xn(tt):
            xn = pxn.tile([128, DIM], BF16, name="xn", tag="xn")
            nc.sync.dma_start(xn[:], xb[ts(tt, 128), :])
            xn_tiles[tt] = xn

        for tt in range(6):
            load_xn(tt)

        with tc.tile_pool(name="sps", bufs=3, space="PSUM") as pps, \
             tc.tile_pool(name="trps", bufs=2, space="PSUM") as ptr, \
             tc.tile_pool(name="a1ps", bufs=1, space="PSUM") as pa1p, \
             tc.tile_pool(name="csps", bufs=1, space="PSUM") as pcsp, \
             tc.tile_pool(name="stmp", bufs=4) as pst:
            a1T = pa1p.tile([128, 8 * 56], F32, name="a1T", tag="a1T")
            pcs = pcsp.tile([56, 1], F32, name="cs", tag="cs")
            p2fs = {}

            def emit_agg(tt):
                for kk in range(8):
                    nc.tensor.matmul(
                        a1T[:, ts(kk, 56)],
                        xn_tiles[tt][:, ts(kk, 128)], u1T[:, ts(tt, 56)],
                        start=(tt == 0), stop=(tt == NT - 1),
                        skip_group_check=True)
                del xn_tiles[tt]
                nc.tensor.matmul(pcs[:], u1T[:, ts(tt, 56)], ones_sb[:],
                                 start=(tt == 0), stop=(tt == NT - 1))

            def emit_p2t(tt):
                ptt = ptr.tile([56, 128], BF16, name="p2t", tag="p2t")
                nc.tensor.transpose(ptt[:], p2fs.pop(tt)[:], idb_v)
                nc.any.tensor_copy(p2T[:, ts(tt, 128)], ptt[:])

            for tt in range(NT + 2):
                if tt < NT:
                    if tt + 6 < NT:
                        load_xn(tt + 6)
                    ps = pps.tile([128, 112], F32, name="s", tag="s")
                    for kk in range(8):
                        nc.tensor.matmul(
                            ps[:], xT(kk)[:, ts(tt, 128)], AB[kk][:],
                            start=(kk == 0), stop=(kk == 7))
                if tt > 1:
                    emit_agg(tt - 2)
                    emit_p2t(tt - 2)
                if tt < NT:
                    nc.scalar.activation(
                        u1T[:, ts(tt, 56)], ps[:, 0:56], AF.Exp)
                    u2 = pst.tile([128, 56], F32, name="u2", tag="u2")
                    nc.scalar.activation(u2[:], ps[:, 56:112], AF.Exp)
                    rs = pst.tile([128, 8], F32, name="rs", tag="rs")
                    nc.vector.reduce_sum(
                        rs[:], u2.rearrange("p (h q) -> p h q", q=P),
                        axis=AX.X)
                    nc.vector.reciprocal(rs[:], rs[:])
                    p2f = pst.tile([128, 56], BF16, name="p2f", tag="p2f")
                    nc.vector.tensor_tensor(
                        out=p2f.rearrange("p (h q) -> p h q", q=P),
                        in0=u2.rearrange("p (h q) -> p h q", q=P),
                        in1=rs[:, :, None].broadcast_to([128, 8, P]),
                        op=AL.mult,
                    )
                    p2fs[tt] = p2f

            nc.vector.reciprocal(rec1[:], pcs[:])
            nc.scalar.copy(a1sb[:], a1T[:])

        xT_stack.close()  # free xTb SBUF

        # ---- dwc + v_agent + attention output -------------------------
        # conv_b is folded into vabd: p2 rows sum to 1 per (token, head),
        # so adding conv_b to the vabd diag blocks makes the fused
        # attention matmul contribute exactly out_att + conv_b.
        # 7 of 9 taps + the stage-2 attention matmul accumulate in PSUM;
        # the two remaining vO taps are fused DVE ops on the extraction
        # path. The first two tap chains are emitted before the va chain
        # so PE stays busy while Act/DVE produce vabd.
        pY = ctx.enter_context(tc.tile_pool(name="Ypool", bufs=1))
        Y = [pY.tile([128, T], BF16, name=f"Y{i}", tag=f"Y{i}")
             for i in range(4)]

        with tc.tile_pool(name="dwcps", bufs=4, space="PSUM") as pdw:
            def emit_taps(i, tc7):
                src_by_kh = (vT[i], vO[i], vT[i + 1])
                off = 1 + tc7 * 512
                pd = pdw.tile([128, 512], F32, name="dwc", tag="dwc")
                first = True
                for kh, kt in ((0, 0), (0, 1), (0, 2), (1, 2),
                               (2, 0), (2, 1), (2, 2)):
                    nc.tensor.matmul(
                        pd[:], taps_v[:, kh * 3 + kt, :],
                        src_by_kh[kh][:, ds(off + kt - 1, 512)],
                        start=first, stop=False)
                    first = False
                return pd

            def emit_finish(i, tc7, pd):
                off = 1 + tc7 * 512
                # stage-2 attention output (+conv_b): last accumulation
                nc.tensor.matmul(
                    pd[:], vabd[:, ts(i, 128)], p2T[:, ts(tc7, 512)],
                    start=False, stop=True)
                yslc = Y[i][:, ts(tc7, 512)]
                nc.vector.scalar_tensor_tensor(
                    out=yslc, in0=vO[i][:, ds(off, 512)],
                    scalar=wcol[:, 4:5], in1=pd[:],
                    op0=AL.mult, op1=AL.add)
                nc.vector.scalar_tensor_tensor(
                    out=yslc, in0=vO[i][:, ds(off - 1, 512)],
                    scalar=wcol[:, 3:4], in1=yslc,
                    op0=AL.mult, op1=AL.add)

            pd00 = emit_taps(0, 0)
            pd01 = emit_taps(0, 1)

            with tc.tile_pool(name="vaps", bufs=1, space="PSUM") as pvap, \
                 tc.tile_pool(name="vtmp", bufs=1) as pvt:
                pva = pvap.tile([56, 512], F32, name="va", tag="va")
                for kk in range(8):
                    nc.tensor.matmul(pva[:, 0:64], a1sb[:, ts(kk, 56)],
                                     wvpa_v[:, kk, ds(64, 64)],
                                     start=(kk == 0), stop=(kk == 7),
                                     skip_group_check=True)
                    nc.tensor.matmul(pva[:, 64:512], a1sb[:, ts(kk, 56)],
                                     wvpb_sb[:, kk, ds(0, 448)],
                                     start=(kk == 0), stop=(kk == 7),
                                     skip_group_check=True)
                van = pvt.tile([56, 512], BF16, name="van", tag="van")
                nc.vector.scalar_tensor_tensor(
                    out=van[:], in0=pva[:], scalar=rec1[:],
                    in1=blob_sb[0:56, OCBT:OCBT + 512],
                    op0=AL.mult, op1=AL.add,
                )
                nc.vector.memset(vabd[:], 0.0)
                for h in range(HL):
                    nc.sync.dma_start(
                        vabd[ds(P * h, P), ds(64 * h, 64)],
                        van[ds(P * h, P), ds(64 * h, 64)])

            emit_finish(0, 0, pd00)
            emit_finish(0, 1, pd01)
            for i in range(4):
                for tc7 in range(2 if i == 0 else 0, NCH):
                    pd = emit_taps(i, tc7)
                    emit_finish(i, tc7, pd)

        # ---- output projection: out = Y^T @ Wo ------------------------
        with tc.tile_pool(name="ostage", bufs=3) as pos, \
             tc.tile_pool(name="ops", bufs=4, space="PSUM") as pop:
            for tt in range(NT):
                po = pop.tile([128, DIM], F32, name="o", tag="o")
                for half in range(2):
                    for k in range(4):
                        nc.tensor.matmul(
                            po[:, ts(half, 512)],
                            Y[k][:, ts(tt, 128)],
                            wop_v[:, k, ts(half, 512)],
                            start=(k == 0), stop=(k == 3))
                ost = pos.tile([128, DIM], BF16, name="ost", tag="ost")
                if tt < NT - 2:
                    _copy(nc.scalar if tt % 2 else nc.vector, ost[:], po[:])
                    nc.gpsimd.dma_start(outp[ts(tt, 128), :], ost[:])
                elif tt == NT - 2:
                    nc.scalar.copy(ost[:], po[:])
                    nc.scalar.dma_start(outp[ts(tt, 128), :], ost[:])
                else:   # last tile: split halves, low-latency HWDGE path
                    nc.vector.tensor_copy(ost[:, 0:512], po[:, 0:512])
                    nc.scalar.copy(ost[:, 512:DIM], po[:, 512:DIM])
                    nc.sync.dma_start(outp[ts(tt, 128), 0:512],
                                      ost[:, 0:512])
                    nc.scalar.dma_start(outp[ts(tt, 128), 512:DIM],
                                        ost[:, 512:DIM])


_NC_CACHE = None


def _get_nc():
    global _NC_CACHE
    if _NC_CACHE is None:
        _NC_CACHE = build_nc()
    return _NC_CACHE


def _prep_core_inputs(x, W_qkv, W_o, conv_w, conv_b):
    bf = ml_dtypes.bfloat16
    ins = []
    # taps[kh*3+kt] = kron(I2, diag(conv_w[:, 0, kh, kt]))
    taps_np = np.zeros((9, 128, 128), dtype=np.float32)
    cw = np.asarray(conv_w, np.float32)
    for kh in range(3):
        for kt in range(3):
            dg = np.diag(cw[:, 0, kh, kt])
            taps_np[kh * 3 + kt, 0:64, 0:64] = dg
            taps_np[kh * 3 + kt, 64:128, 64:128] = dg
    fblob = np.zeros((128, 138), np.float32)
    fblob[:, 0:128] = np.eye(128, dtype=np.float32)
    fblob[:, 128] = np.tile(conv_b, 2)
    for kh in range(3):
        for kt in range(3):
            fblob[:, 129 + kh * 3 + kt] = np.tile(cw[:, 0, kh, kt], 2)

    def pack(w, k):  # [k*128, c] -> [128, k, c]
        c = w.shape[1]
        return np.ascontiguousarray(
            w.reshape(k, 128, c).transpose(1, 0, 2))

    for c in range(8):
        b, g = c // 2, c % 2
        wq = W_qkv[:, 512 * g:512 * g + 512]
        wk = W_qkv[:, 1024 + 512 * g:1024 + 512 * g + 512]
        wv10 = np.zeros((DIM, 640), np.float32)
        for s in range(10):
            h = 8 * g - 1 + s
            if 0 <= h < 16:
                wv10[:, 64 * s:64 * s + 64] = \
                    W_qkv[:, 2048 + 64 * h:2048 + 64 * h + 64]
        blob = np.empty((128, BLOBW), np.float32)
        blob[:, OWQN:OWQN + 4096] = pack(wq, 8).reshape(128, 4096)
        blob[:, OWKT:OWKT + 4096] = pack(
            np.ascontiguousarray(wk.T), 4).reshape(128, 4096)
        blob[:, OWQT:OWQT + 4096] = pack(
            np.ascontiguousarray(wq.T), 4).reshape(128, 4096)
        blob[:, OWOP:OWOP + 4096] = pack(
            np.ascontiguousarray(W_o[512 * g:512 * g + 512, :]),
            4).reshape(128, 4096)
        blob[:, OTAPS:OTAPS + 1152] = taps_np.transpose(1, 0, 2).reshape(
            128, 1152)
        blob[:, OCBT:OCBT + 512] = np.tile(conv_b, 8)[None, :]
        blob[:, OIDB:OIDB + 128] = np.eye(128, dtype=np.float32)
        ins.append({
            "xbT": np.ascontiguousarray(x[b].T).astype(bf),
            "xb": np.ascontiguousarray(x[b]).astype(bf),
            "wvpa": pack(wv10[:, 0:128], 8).reshape(128, 1024).astype(bf),
            "wvpb": pack(wv10[:, 128:640], 8).astype(bf),
            "blob": blob.astype(bf),
            "fblob": fblob,
        })
    return ins


def kernel(x, W_qkv, W_o, b_o, conv_w, conv_b, _run_kwargs=None):
    x = np.asarray(x, np.float32)
    W_qkv = np.asarray(W_qkv, np.float32)
    W_o = np.asarray(W_o, np.float32)
    b_o = np.asarray(b_o, np.float32)
    conv_w = np.asarray(conv_w, np.float32)
    conv_b = np.asarray(conv_b, np.float32)

    ins = _prep_core_inputs(x, W_qkv, W_o, conv_w, conv_b)

    nc = _get_nc()
    res = bass_utils.run_bass_kernel_spmd(
        nc, ins, core_ids=list(range(8)), **(_run_kwargs or {}))
    outs = [r["outp"] for r in res.results]
    B = x.shape[0]
    full = np.empty((B, T, DIM), np.float32)
    for b in range(B):
        full[b] = (outs[2 * b].astype(np.float32)
                   + outs[2 * b + 1].astype(np.float32) + b_o[None, :])
    if _run_kwargs:
        kernel.last_results = res
    return full
